# revision 12
# baseline (speedup 1.0000x reference)
"""Trainium2 Bass kernel for nn_BDHGPURefStabilized.

Model (per batch element b, scan over T steps):
    v_t   = token_emb[tok_t]                         # [D]
    xt    = 0.97*x + v_t @ Dx.T                      # [N]
    xt    = xt / (sum|xt| + 1e-6)
    xt    = where(xt > 0.02*max(xt), xt, 0)
    a*    = rho @ xt                                 # fast-weight read [D]
    y     = LN(a*) @ Dy.T                            # [N]
    yt    = relu(y) * relu(xt)
    v*_t  = LN(yt @ E.T)                             # output row [D]
    rho   = 0.97*(rho + v_t (x) xt)                  # rank-1 fast-weight update

Kernel strategy (8 NeuronCores, data-parallel over batch B=8, one batch
element per core, zero collectives):

 - Split the computation into a minimal serial spine and a fully batched
   output chain.  The spine is the only true recurrence: with the
   rescaling w_t = xt_t/0.97 and host-prescaled P~ = (v @ Dx.T)/0.97,
       w_{t+1} = (w_t > 0.02 max(w_t)) * w_t / sum|w_t| + P~_{t+1}
   and the masked-normalized history is x_t = w_{t+1} - P~_{t+1}.
   Per step: two DVE free-axis reductions, two gpsimd partition_all_reduce
   ops (cross-partition max / sum, result broadcast to all partitions),
   and three DVE element-wise ops.  History (x_t and relu(x_t), fp16) is
   written by the otherwise-idle ACT engine off the critical path.
 - The output chain is a pure function of the history, so it runs once,
   batched over all T as large matmuls: G = Xh^T Xh (Gram vs history),
   strict-causal mask, A = G^T @ (0.97^{-s} v_s) with the 0.97^t factor
   folded into a per-row LayerNorm epsilon, then LN, y = LN(A) @ Dy.T,
   yt = relu(y)*relu(x), u = yt @ E.T, out = LN(u) — all LayerNorms
   batched 128 rows/op via bn_stats.  Output-path matmuls run fp16
   (PSUM accumulates fp32); the spine stays fp32.
 - This takes the Tensor-engine sequencer from ~28k instructions (the
   previous per-step formulation) to ~120, and the per-step critical
   path from ~4.5us to ~0.6us.

Output per core: [T, 128] fp32 rows; host stacks [B, T, D].
"""

from contextlib import ExitStack

import numpy as np

import concourse.bass as bass
import concourse.bacc as bacc
import concourse.tile as tile
from concourse import bass_isa, mybir

F32 = mybir.dt.float32
F16 = mybir.dt.float16
AX = mybir.AxisListType
OP = mybir.AluOpType
AF = mybir.ActivationFunctionType
RED = bass_isa.ReduceOp

N, D, V = 2048, 128, 131072
C = N // 128  # 16 column-chunks of n; n = c*128 + j
U_DECAY, X_DECAY, THR = 0.97, 0.97, 0.02


def scan_program(tc, outs, ins, T):
    nc = tc.nc
    ctx = ExitStack()
    TC = T // 128          # t-chunks (2 for T=256)
    W32 = N + T + TC       # packed f32 input width
    W16 = 2 * N + TC * 128 + TC * T + 128  # packed f16 input width

    with ctx:
        wpool = ctx.enter_context(tc.tile_pool(name="weights", bufs=1))
        spool = ctx.enter_context(tc.tile_pool(name="step", bufs=3))
        scal = ctx.enter_context(tc.tile_pool(name="scal", bufs=4))

        B32 = wpool.tile([128, W32], F32, tag="B32")
        B16 = wpool.tile([128, W16], F16, tag="B16")
        nc.sync.dma_start(out=B32, in_=ins["B32"])
        nc.sync.dma_start(out=B16, in_=ins["B16"])
        DxT = B32[:, 0:N]                          # [d, n]/0.97
        Vt = B32[:, N:N + T]                       # [d, t]
        eps2 = B32[:, N + T:N + T + TC]            # LN(A) eps per t-chunk
        DyTr = B16[:, 0:N].rearrange("p (c j) -> p c j", c=C)      # [d,(c,j)]
        ETr = B16[:, N:2 * N].rearrange("p (c j) -> p c j", c=C)   # [j,(c,d)]
        Vh = B16[:, 2 * N:2 * N + TC * 128].rearrange(
            "p (s j) -> p s j", s=TC)                              # [s,(sc,d)]
        mask16 = B16[:, 2 * N + TC * 128:2 * N + TC * 128 + TC * T].rearrange(
            "p (s j) -> p s j", s=TC)                              # [s,(sc,t)]
        idn16 = B16[:, 2 * N + TC * 128 + TC * T:]                 # [128,128] I

        # persistent SBUF state.  Spine state is laid out with one column per
        # step (zero buffer reuse): every spine instruction then carries at
        # most ONE semaphore wait, so bacc's multi-wait splitting never
        # inserts SEQ-blocking EventSemaphore instructions into the loop.
        P_sb = wpool.tile([128, T, C], F32, tag="P_sb")    # P~ = v@Dx.T/0.97
        Xh = wpool.tile([128, C, T], F16, tag="Xh")        # x_t history
        W16h = wpool.tile([128, C, T], F16, tag="W16h")    # relu(x_t) history
        Wsp = wpool.tile([128, T, C], F32, tag="Wsp")      # spine state u_t
        Ysp = wpool.tile([128, T, C], F32, tag="Ysp")      # masked u_t
        red2T = wpool.tile([128, 2, T], F32, tag="red2T")  # partial reduces
        ar2T = wpool.tile([128, 2, T], F32, tag="ar2T")    # all-reduced S, m
        thrT = wpool.tile([128, 1, T], F32, tag="thrT")
        rall = wpool.tile([128, T], F32, tag="rall")       # 1/S_t (bulk)

        # ---- P~ = DxT~ @ V (device-side, fp32) ----
        # PSUM evacuation on DVE (not ACT) so the spine's P_sb readers wait
        # on the DVE semaphore only (coalesces with their other DVE waits).
        with tc.tile_pool(name="psetup", bufs=2, space="PSUM") as psetup:
            for c in range(C):
                p_ps = psetup.tile([128, T], F32, tag="pp")
                nc.tensor.matmul(
                    p_ps, DxT[:, c * 128:(c + 1) * 128], Vt,
                    start=True, stop=True,
                )
                nc.vector.tensor_copy(P_sb[:, :, c], p_ps)

        # ---- serial spine (unnormalized u-space) ----
        # The step map  u -> S*P~_next + mask(u)*u  (S = sum|u|) is
        # homogeneous of degree 1, so running it unnormalized preserves the
        # exact direction of the reference state while removing the
        # reciprocal from the recurrence: u_{t+1} = S_t * P~_{t+1} + y_t.
        # An exact power-of-two rescale every RESC steps bounds |u| in fp32.
        # Histories are normalized in bulk afterwards via y_t / S_t.
        # Pool stays PartitionAllReduce-only (any standard gpsimd op in the
        # loop would force a Q7 library reload per step).
        RESC = 4
        K4 = float(2.0 ** -29)
        for t in range(T):
            u = Wsp[:, t, :] if t > 0 else P_sb[:, 0, :]
            nc.vector.tensor_reduce(
                out=red2T[:, 1, t:t + 1], in_=u, axis=AX.X, op=OP.max)
            nc.vector.tensor_reduce(
                out=red2T[:, 0, t:t + 1], in_=u, axis=AX.X, op=OP.add,
                apply_absolute_value=True)
            nc.gpsimd.partition_all_reduce(
                ar2T[:, 1, t:t + 1], red2T[:, 1, t:t + 1], 128, RED.max)
            nc.gpsimd.partition_all_reduce(
                ar2T[:, 0, t:t + 1], red2T[:, 0, t:t + 1], 128, RED.add)
            thr = thrT[:, 0, t:t + 1]
            nc.vector.tensor_scalar(
                out=thr, in0=ar2T[:, 1, t:t + 1], scalar1=float(THR),
                scalar2=None, op0=OP.mult)
            y = Ysp[:, t, :]
            nc.vector.scalar_tensor_tensor(
                out=y, in0=u, scalar=thr, in1=u, op0=OP.is_gt, op1=OP.mult)
            if t + 1 < T:
                nc.vector.scalar_tensor_tensor(
                    out=Wsp[:, t + 1, :], in0=P_sb[:, t + 1, :],
                    scalar=ar2T[:, 0, t:t + 1], in1=y,
                    op0=OP.mult, op1=OP.add)
                if (t + 1) % RESC == 0:
                    nc.vector.tensor_scalar(
                        out=Wsp[:, t + 1, :], in0=Wsp[:, t + 1, :],
                        scalar1=K4, scalar2=None, op0=OP.mult)

        # bulk history normalization: x_t = y_t / S_t (fp16), relu on ACT
        nc.vector.reciprocal(out=rall, in_=ar2T[:, 0, :])
        for c in range(C):
            nc.vector.tensor_tensor(
                out=Xh[:, c, :], in0=Ysp[:, :, c], in1=rall, op=OP.mult)
            nc.scalar.activation(
                out=W16h[:, c, :], in_=Xh[:, c, :], func=AF.Relu)

        # ---- batched output chain ----
        # G[s, t] = x_s . x_t   (strict-causal masked, fp16)
        G16 = wpool.tile([128, TC, T], F16, tag="G16")
        with tc.tile_pool(name="pG", bufs=2, space="PSUM") as pG:
            for sc in range(TC):
                g_ps = pG.tile([128, T], F32, tag="g")
                for c in range(C):
                    nc.tensor.matmul(
                        g_ps, Xh[:, c, sc * 128:(sc + 1) * 128], Xh[:, c, :],
                        start=(c == 0), stop=(c == C - 1),
                    )
                nc.vector.tensor_tensor(
                    out=G16[:, sc, :], in0=g_ps, in1=mask16[:, sc, :],
                    op=OP.mult)

        # A[t, d] = sum_s G[s, t] Vh[s, d];  LnA = rowwise-LN(A, eps_t)
        LnAT = wpool.tile([128, T], F16, tag="LnAT")     # [d, t]
        with tc.tile_pool(name="pA", bufs=2, space="PSUM") as pA, \
                tc.tile_pool(name="pT", bufs=2, space="PSUM") as pT:
            for tcn in range(TC):
                a_ps = pA.tile([128, 128], F32, tag="a")
                for sc in range(TC):
                    nc.tensor.matmul(
                        a_ps, G16[:, sc, tcn * 128:(tcn + 1) * 128],
                        Vh[:, sc, :], start=(sc == 0), stop=(sc == TC - 1),
                    )
                lnA = _layernorm_rows(
                    tc, spool, scal, a_ps, eps2[:, tcn:tcn + 1], F16)
                t_ps = pT.tile([128, 128], F16, tag="t")
                nc.tensor.transpose(t_ps, lnA, idn16)
                nc.scalar.copy(LnAT[:, tcn * 128:(tcn + 1) * 128], t_ps)

        # yT[n, t] = Dy @ LnA^T;  yt = relu(yT) * relu(x)  (fp16)
        yt_sb = wpool.tile([128, C, T], F16, tag="yt")
        with tc.tile_pool(name="pY", bufs=3, space="PSUM") as pY:
            for c in range(C):
                y_ps = pY.tile([128, T], F32, tag="y")
                nc.tensor.matmul(y_ps, DyTr[:, c, :], LnAT,
                                 start=True, stop=True)
                if c % 2 == 0:
                    # DVE: fused relu+mask-multiply straight from PSUM
                    nc.vector.scalar_tensor_tensor(
                        out=yt_sb[:, c, :], in0=y_ps, scalar=0.0,
                        in1=W16h[:, c, :], op0=OP.max, op1=OP.mult)
                else:
                    # ACT evacuates with relu; DVE multiplies fp16 (2x mode)
                    ry = spool.tile([128, T], F16, tag="ry")
                    nc.scalar.activation(out=ry, in_=y_ps, func=AF.Relu)
                    nc.vector.tensor_tensor(
                        out=yt_sb[:, c, :], in0=ry, in1=W16h[:, c, :],
                        op=OP.mult)

        # u[t, d] = sum_n yt[n, t] E[d, n];  out = rowwise-LN(u)
        with tc.tile_pool(name="pU", bufs=2, space="PSUM") as pU:
            for tcn in range(TC):
                u_ps = pU.tile([128, 128], F32, tag="u")
                for c in range(C):
                    nc.tensor.matmul(
                        u_ps, yt_sb[:, c, tcn * 128:(tcn + 1) * 128],
                        ETr[:, c, :], start=(c == 0), stop=(c == C - 1),
                    )
                o_sb = _layernorm_rows(tc, spool, scal, u_ps, 1e-6, F32)
                nc.sync.dma_start(
                    out=outs["out"][tcn * 128:(tcn + 1) * 128, :], in_=o_sb)


def _layernorm_rows(tc, spool, scal, rows_ps, eps, out_dtype):
    """Row-wise LayerNorm of a [128, 128] PSUM tile (torch-style: ddof=1,
    eps added to std).  ``eps`` is a float or a [128, 1] AP (per-row).
    Returns a [128, 128] SBUF tile of out_dtype."""
    nc = tc.nc
    stats = scal.tile([128, 6], F32, tag="ln_stats")
    mv = scal.tile([128, 2], F32, tag="ln_mv")
    nc.vector.bn_stats(out=stats, in_=rows_ps)
    nc.vector.bn_aggr(out=mv, in_=stats)
    sd = scal.tile([128, 2], F32, tag="ln_sd")
    nc.scalar.activation(
        out=sd[:, 0:1], in_=mv[:, 1:2], func=AF.Sqrt,
        scale=float(D) / (D - 1))
    if isinstance(eps, float):
        nc.vector.tensor_scalar(
            out=sd[:, 1:2], in0=sd[:, 0:1], scalar1=eps, scalar2=None,
            op0=OP.add)
    else:
        nc.vector.tensor_scalar(
            out=sd[:, 1:2], in0=sd[:, 0:1], scalar1=eps, scalar2=None,
            op0=OP.add)
    rstd = scal.tile([128, 1], F32, tag="ln_rstd")
    nc.vector.reciprocal(out=rstd, in_=sd[:, 1:2])
    out = spool.tile([128, 128], out_dtype, tag=f"ln_out_{out_dtype}")
    nc.vector.tensor_scalar(
        out=out, in0=rows_ps, scalar1=mv[:, 0:1], scalar2=rstd,
        op0=OP.subtract, op1=OP.mult)
    return out


# ----------------------------------------------------------------------------
# host side
# ----------------------------------------------------------------------------

def _host_prep_shared(E, Dx, Dy, T):
    """Packed B32/B16 templates (per-core slots for Vt/Vh left zero)."""
    TC = T // 128
    W32 = N + T + TC
    W16 = 2 * N + TC * 128 + TC * T + 128
    B32 = np.zeros((128, W32), dtype=np.float32)
    B32[:, 0:N] = Dx.T / X_DECAY
    for tcn in range(TC):
        ts = tcn * 128 + np.arange(128, dtype=np.float64)
        B32[:, N + T + tcn] = (1e-6 * U_DECAY ** (-ts)).astype(np.float32)
    B16 = np.zeros((128, W16), dtype=np.float16)
    B16[:, 0:N] = Dy.reshape(C, 128, D).transpose(2, 0, 1).reshape(128, N)
    B16[:, N:2 * N] = E.reshape(D, C, 128).transpose(2, 1, 0).reshape(128, N)
    # strict-causal mask: mask[sc][i, t] = (sc*128 + i) < t
    s_all = np.arange(T)[:, None]
    t_all = np.arange(T)[None, :]
    m = (s_all < t_all).astype(np.float16)           # [s, t]
    B16[:, 2 * N + TC * 128:2 * N + TC * 128 + TC * T] = (
        m.reshape(TC, 128, T).transpose(1, 0, 2).reshape(128, TC * T))
    B16[:, 2 * N + TC * 128 + TC * T:] = np.eye(128, dtype=np.float16)
    return B32, B16


def _host_prep_core(B32t, B16t, token_emb, tokens_b, T):
    TC = T // 128
    B32 = B32t.copy()
    B16 = B16t.copy()
    V_all = token_emb[tokens_b].astype(np.float32)         # [T, 128]
    B32[:, N:N + T] = V_all.T
    decay = (U_DECAY ** (-np.arange(T, dtype=np.float64))).astype(np.float32)
    Vh_flat = (V_all * decay[:, None]).astype(np.float16)  # [T, 128]
    B16[:, 2 * N:2 * N + TC * 128] = (
        Vh_flat.reshape(TC, 128, 128).transpose(1, 0, 2).reshape(128, TC * 128))
    return dict(B32=B32, B16=B16)


_PROGRAM_CACHE = {}
RUN_KWARGS = {}      # extra kwargs forwarded to run_bass_kernel_spmd
LAST_RESULTS = None  # BassKernelResults of the most recent kernel() call


def _build(T):
    key = T
    if key in _PROGRAM_CACHE:
        return _PROGRAM_CACHE[key]
    TC = T // 128
    W32 = N + T + TC
    W16 = 2 * N + TC * 128 + TC * T + 128
    nc = bacc.Bacc("TRN2")
    ins = {
        "B32": nc.dram_tensor("B32", [128, W32], F32, kind="ExternalInput").ap(),
        "B16": nc.dram_tensor("B16", [128, W16], F16, kind="ExternalInput").ap(),
    }
    outs = {
        "out": nc.dram_tensor("out", [T, D], F32, kind="ExternalOutput").ap(),
    }
    with tile.TileContext(nc) as tc:
        scan_program(tc, outs, ins, T)
    nc.compile()
    _PROGRAM_CACHE[key] = (nc, ins, outs)
    return _PROGRAM_CACHE[key]


def kernel(E, Dx, Dy, token_emb, tokens):
    from concourse.bass_utils import run_bass_kernel_spmd

    E = np.asarray(E, dtype=np.float32)
    Dx = np.asarray(Dx, dtype=np.float32)
    Dy = np.asarray(Dy, dtype=np.float32)
    token_emb = np.asarray(token_emb, dtype=np.float32)
    tokens = np.asarray(tokens)
    B, T = tokens.shape

    nc, ins, outs = _build(T)
    B32t, B16t = _host_prep_shared(E, Dx, Dy, T)
    in_maps = [
        _host_prep_core(B32t, B16t, token_emb, tokens[b], T) for b in range(B)
    ]

    res = run_bass_kernel_spmd(nc, in_maps, core_ids=list(range(B)), **RUN_KWARGS)
    global LAST_RESULTS
    LAST_RESULTS = res
    out = np.stack([r["out"] for r in res.results])  # [B, T, 128]
    return out.astype(np.float32)


# revision 20
# speedup vs baseline: 1.0858x; 1.0858x over previous
"""Trainium2 Bass kernel for nn_BDHGPURefStabilized.

Model (per batch element b, scan over T steps):
    v_t   = token_emb[tok_t]                         # [D]
    xt    = 0.97*x + v_t @ Dx.T                      # [N]
    xt    = xt / (sum|xt| + 1e-6)
    xt    = where(xt > 0.02*max(xt), xt, 0)
    a*    = rho @ xt                                 # fast-weight read [D]
    y     = LN(a*) @ Dy.T                            # [N]
    yt    = relu(y) * relu(xt)
    v*_t  = LN(yt @ E.T)                             # output row [D]
    rho   = 0.97*(rho + v_t (x) xt)                  # rank-1 fast-weight update

Kernel strategy (8 NeuronCores, data-parallel over batch B=8, one batch
element per core, zero collectives):

 - Split the computation into a minimal serial spine and a fully batched
   output chain.  The spine is the only true recurrence: with the
   rescaling w_t = xt_t/0.97 and host-prescaled P~ = (v @ Dx.T)/0.97,
       w_{t+1} = (w_t > 0.02 max(w_t)) * w_t / sum|w_t| + P~_{t+1}
   and the masked-normalized history is x_t = w_{t+1} - P~_{t+1}.
   Per step: two DVE free-axis reductions, two gpsimd partition_all_reduce
   ops (cross-partition max / sum, result broadcast to all partitions),
   and three DVE element-wise ops.  History (x_t and relu(x_t), fp16) is
   written by the otherwise-idle ACT engine off the critical path.
 - The output chain is a pure function of the history, so it runs once,
   batched over all T as large matmuls: G = Xh^T Xh (Gram vs history),
   strict-causal mask, A = G^T @ (0.97^{-s} v_s) with the 0.97^t factor
   folded into a per-row LayerNorm epsilon, then LN, y = LN(A) @ Dy.T,
   yt = relu(y)*relu(x), u = yt @ E.T, out = LN(u) — all LayerNorms
   batched 128 rows/op via bn_stats.  Output-path matmuls run fp16
   (PSUM accumulates fp32); the spine stays fp32.
 - This takes the Tensor-engine sequencer from ~28k instructions (the
   previous per-step formulation) to ~120, and the per-step critical
   path from ~4.5us to ~0.6us.

Output per core: [T, 128] fp32 rows; host stacks [B, T, D].
"""

from contextlib import ExitStack

import numpy as np

import concourse.bass as bass
import concourse.bacc as bacc
import concourse.tile as tile
from concourse import bass_isa, mybir

F32 = mybir.dt.float32
F16 = mybir.dt.float16
AX = mybir.AxisListType
OP = mybir.AluOpType
AF = mybir.ActivationFunctionType
RED = bass_isa.ReduceOp

N, D, V = 2048, 128, 131072
C = N // 128  # 16 column-chunks of n; n = c*128 + j
U_DECAY, X_DECAY, THR = 0.97, 0.97, 0.02

# spine emission variant (selected by TimelineSim sweep)
SPINE = {
    "thr_mode": "post",     # "pre" | "post"
    "pool_order": "max_first",  # "max_first" | "sum_first"
    "red_order": "max_first",   # "max_first" | "sum_first"
    "hist": "bulk",         # "act" | "bulk"
}


def scan_program(tc, outs, ins, T):
    nc = tc.nc
    ctx = ExitStack()
    TC = T // 128          # t-chunks (2 for T=256)
    W32 = N + T + TC       # packed f32 input width
    W16 = 2 * N + TC * 128 + TC * T + 128  # packed f16 input width

    with ctx:
        wpool = ctx.enter_context(tc.tile_pool(name="weights", bufs=1))
        spool = ctx.enter_context(tc.tile_pool(name="step", bufs=3))
        scal = ctx.enter_context(tc.tile_pool(name="scal", bufs=4))

        B32 = wpool.tile([128, W32], F32, tag="B32")
        B16 = wpool.tile([128, W16], F16, tag="B16")
        nc.sync.dma_start(out=B32, in_=ins["B32"])
        nc.sync.dma_start(out=B16, in_=ins["B16"])
        DxT = B32[:, 0:N]                          # [d, n]/0.97
        Vt = B32[:, N:N + T]                       # [d, t]
        eps2 = B32[:, N + T:N + T + TC]            # LN(A) eps per t-chunk
        DyTr = B16[:, 0:N].rearrange("p (c j) -> p c j", c=C)      # [d,(c,j)]
        ETr = B16[:, N:2 * N].rearrange("p (c j) -> p c j", c=C)   # [j,(c,d)]
        Vh = B16[:, 2 * N:2 * N + TC * 128].rearrange(
            "p (s j) -> p s j", s=TC)                              # [s,(sc,d)]
        mask16 = B16[:, 2 * N + TC * 128:2 * N + TC * 128 + TC * T].rearrange(
            "p (s j) -> p s j", s=TC)                              # [s,(sc,t)]
        idn16 = B16[:, 2 * N + TC * 128 + TC * T:]                 # [128,128] I

        # persistent SBUF state.  Spine state is laid out with one column per
        # step (zero buffer reuse): every spine instruction then carries at
        # most ONE semaphore wait, so bacc's multi-wait splitting never
        # inserts SEQ-blocking EventSemaphore instructions into the loop.
        P_sb = wpool.tile([128, T, C], F32, tag="P_sb")    # P~ = v@Dx.T/0.97
        Xh = wpool.tile([128, C, T], F16, tag="Xh")        # x_t history
        W16h = wpool.tile([128, C, T], F16, tag="W16h")    # relu(x_t) history
        Wsp = wpool.tile([128, T, C], F32, tag="Wsp")      # spine state u_t
        Ysp = wpool.tile([128, T, C], F32, tag="Ysp")      # masked u_t
        red2T = wpool.tile([128, 2, T], F32, tag="red2T")  # partial reduces
        ar2T = wpool.tile([128, 2, T], F32, tag="ar2T")    # all-reduced S, m
        thrT = wpool.tile([128, 1, T], F32, tag="thrT")
        invT = wpool.tile([128, 1, T], F32, tag="invT")
        rall = wpool.tile([128, T], F32, tag="rall")       # 1/S_t (bulk)

        # ---- P~ = DxT~ @ V (device-side, fp32) ----
        # PSUM evacuation on DVE (not ACT) so the spine's P_sb readers wait
        # on the DVE semaphore only (coalesces with their other DVE waits).
        with tc.tile_pool(name="psetup", bufs=2, space="PSUM") as psetup:
            for c in range(C):
                p_ps = psetup.tile([128, T], F32, tag="pp")
                nc.tensor.matmul(
                    p_ps, DxT[:, c * 128:(c + 1) * 128], Vt,
                    start=True, stop=True,
                )
                nc.vector.tensor_copy(P_sb[:, :, c], p_ps)

        # ---- serial spine ----
        # The 0.02 threshold scale is applied to the PER-PARTITION partial
        # maxima before the cross-partition allreduce (exact: fl(0.02*x) is
        # monotone, so max_p fl(0.02*m_p) == fl(0.02*max_p m_p)), so the
        # Pool allreduce returns thr directly and the mask STT carries a
        # single Pool-sem wait.  Pool stays PartitionAllReduce-only (any
        # standard gpsimd op in the loop would force a Q7 library reload
        # per step).  Histories are normalized in bulk after the loop.
        for t in range(T):
            u = Wsp[:, t, :] if t > 0 else P_sb[:, 0, :]
            pmax = red2T[:, 1, t:t + 1]
            psum = red2T[:, 0, t:t + 1]
            armax = ar2T[:, 1, t:t + 1]
            arsum = ar2T[:, 0, t:t + 1]
            thrp = thrT[:, 0, t:t + 1]
            invs = invT[:, 0, t:t + 1]
            y = Ysp[:, t, :]

            def emit_redmax():
                nc.vector.tensor_reduce(
                    out=pmax, in_=u, axis=AX.X, op=OP.max)
                if SPINE["thr_mode"] == "pre":
                    # pre-scale by 0.02 (exact: fl(0.02*x) is monotone) so
                    # the Pool allreduce returns thr directly
                    nc.vector.tensor_scalar(
                        out=thrp, in0=pmax, scalar1=float(THR),
                        scalar2=None, op0=OP.mult)

            def emit_redsum():
                nc.vector.tensor_reduce(
                    out=psum, in_=u, axis=AX.X, op=OP.add,
                    apply_absolute_value=True)

            def emit_par_max():
                nc.gpsimd.partition_all_reduce(
                    armax, thrp if SPINE["thr_mode"] == "pre" else pmax,
                    128, RED.max)

            def emit_par_sum():
                nc.gpsimd.partition_all_reduce(arsum, psum, 128, RED.add)

            if SPINE["red_order"] == "max_first":
                emit_redmax()
                emit_redsum()
            else:
                emit_redsum()
                emit_redmax()
            if SPINE["pool_order"] == "max_first":
                emit_par_max()
                emit_par_sum()
            else:
                emit_par_sum()
                emit_par_max()
            if SPINE["thr_mode"] == "post":
                nc.vector.tensor_scalar(
                    out=thrp, in0=armax, scalar1=float(THR),
                    scalar2=None, op0=OP.mult)
                thr_ap = thrp
            else:
                thr_ap = armax
            nc.vector.scalar_tensor_tensor(
                out=y, in0=u, scalar=thr_ap, in1=u,
                op0=OP.is_gt, op1=OP.mult)
            nc.vector.reciprocal(out=invs, in_=arsum)
            if t + 1 < T:
                nc.vector.scalar_tensor_tensor(
                    out=Wsp[:, t + 1, :], in0=y, scalar=invs,
                    in1=P_sb[:, t + 1, :], op0=OP.mult, op1=OP.add)
            if SPINE["hist"] == "act":
                nc.scalar.activation(
                    out=Xh[:, :, t], in_=y, func=AF.Copy, scale=invs)
                nc.scalar.activation(
                    out=W16h[:, :, t], in_=y, func=AF.Relu, scale=invs)
        if SPINE["hist"] == "bulk":
            nc.vector.reciprocal(out=rall, in_=ar2T[:, 0, :])
            for c in range(C):
                nc.vector.tensor_tensor(
                    out=Xh[:, c, :], in0=Ysp[:, :, c], in1=rall, op=OP.mult)
                nc.scalar.activation(
                    out=W16h[:, c, :], in_=Xh[:, c, :], func=AF.Relu)

        # ---- batched output chain ----
        # G[s, t] = x_s . x_t   (strict-causal masked, fp16)
        G16 = wpool.tile([128, TC, T], F16, tag="G16")
        with tc.tile_pool(name="pG", bufs=2, space="PSUM") as pG:
            for sc in range(TC):
                g_ps = pG.tile([128, T], F32, tag="g")
                for c in range(C):
                    nc.tensor.matmul(
                        g_ps, Xh[:, c, sc * 128:(sc + 1) * 128], Xh[:, c, :],
                        start=(c == 0), stop=(c == C - 1),
                    )
                nc.vector.tensor_tensor(
                    out=G16[:, sc, :], in0=g_ps, in1=mask16[:, sc, :],
                    op=OP.mult)

        # A[t, d] = sum_s G[s, t] Vh[s, d];  LnA = rowwise-LN(A, eps_t)
        LnAT = wpool.tile([128, T], F16, tag="LnAT")     # [d, t]
        with tc.tile_pool(name="pA", bufs=2, space="PSUM") as pA, \
                tc.tile_pool(name="pT", bufs=2, space="PSUM") as pT:
            for tcn in range(TC):
                a_ps = pA.tile([128, 128], F32, tag="a")
                for sc in range(TC):
                    nc.tensor.matmul(
                        a_ps, G16[:, sc, tcn * 128:(tcn + 1) * 128],
                        Vh[:, sc, :], start=(sc == 0), stop=(sc == TC - 1),
                    )
                lnA = _layernorm_rows(
                    tc, spool, scal, a_ps, eps2[:, tcn:tcn + 1], F16)
                t_ps = pT.tile([128, 128], F16, tag="t")
                nc.tensor.transpose(t_ps, lnA, idn16)
                nc.scalar.copy(LnAT[:, tcn * 128:(tcn + 1) * 128], t_ps)

        # yT[n, t] = Dy @ LnA^T;  yt = relu(yT) * relu(x)  (fp16)
        yt_sb = wpool.tile([128, C, T], F16, tag="yt")
        with tc.tile_pool(name="pY", bufs=3, space="PSUM") as pY:
            for c in range(C):
                y_ps = pY.tile([128, T], F32, tag="y")
                nc.tensor.matmul(y_ps, DyTr[:, c, :], LnAT,
                                 start=True, stop=True)
                if c % 2 == 0:
                    # DVE: fused relu+mask-multiply straight from PSUM
                    nc.vector.scalar_tensor_tensor(
                        out=yt_sb[:, c, :], in0=y_ps, scalar=0.0,
                        in1=W16h[:, c, :], op0=OP.max, op1=OP.mult)
                else:
                    # ACT evacuates with relu; DVE multiplies fp16 (2x mode)
                    ry = spool.tile([128, T], F16, tag="ry")
                    nc.scalar.activation(out=ry, in_=y_ps, func=AF.Relu)
                    nc.vector.tensor_tensor(
                        out=yt_sb[:, c, :], in0=ry, in1=W16h[:, c, :],
                        op=OP.mult)

        # u[t, d] = sum_n yt[n, t] E[d, n];  out = rowwise-LN(u)
        with tc.tile_pool(name="pU", bufs=2, space="PSUM") as pU:
            for tcn in range(TC):
                u_ps = pU.tile([128, 128], F32, tag="u")
                for c in range(C):
                    nc.tensor.matmul(
                        u_ps, yt_sb[:, c, tcn * 128:(tcn + 1) * 128],
                        ETr[:, c, :], start=(c == 0), stop=(c == C - 1),
                    )
                o_sb = _layernorm_rows(tc, spool, scal, u_ps, 1e-6, F32)
                nc.sync.dma_start(
                    out=outs["out"][tcn * 128:(tcn + 1) * 128, :], in_=o_sb)


def _layernorm_rows(tc, spool, scal, rows_ps, eps, out_dtype):
    """Row-wise LayerNorm of a [128, 128] PSUM tile (torch-style: ddof=1,
    eps added to std).  ``eps`` is a float or a [128, 1] AP (per-row).
    Returns a [128, 128] SBUF tile of out_dtype."""
    nc = tc.nc
    stats = scal.tile([128, 6], F32, tag="ln_stats")
    mv = scal.tile([128, 2], F32, tag="ln_mv")
    nc.vector.bn_stats(out=stats, in_=rows_ps)
    nc.vector.bn_aggr(out=mv, in_=stats)
    sd = scal.tile([128, 2], F32, tag="ln_sd")
    nc.scalar.activation(
        out=sd[:, 0:1], in_=mv[:, 1:2], func=AF.Sqrt,
        scale=float(D) / (D - 1))
    if isinstance(eps, float):
        nc.vector.tensor_scalar(
            out=sd[:, 1:2], in0=sd[:, 0:1], scalar1=eps, scalar2=None,
            op0=OP.add)
    else:
        nc.vector.tensor_scalar(
            out=sd[:, 1:2], in0=sd[:, 0:1], scalar1=eps, scalar2=None,
            op0=OP.add)
    rstd = scal.tile([128, 1], F32, tag="ln_rstd")
    nc.vector.reciprocal(out=rstd, in_=sd[:, 1:2])
    out = spool.tile([128, 128], out_dtype, tag=f"ln_out_{out_dtype}")
    nc.vector.tensor_scalar(
        out=out, in0=rows_ps, scalar1=mv[:, 0:1], scalar2=rstd,
        op0=OP.subtract, op1=OP.mult)
    return out


# ----------------------------------------------------------------------------
# host side
# ----------------------------------------------------------------------------

def _host_prep_shared(E, Dx, Dy, T):
    """Packed B32/B16 templates (per-core slots for Vt/Vh left zero)."""
    TC = T // 128
    W32 = N + T + TC
    W16 = 2 * N + TC * 128 + TC * T + 128
    B32 = np.zeros((128, W32), dtype=np.float32)
    B32[:, 0:N] = Dx.T / X_DECAY
    for tcn in range(TC):
        ts = tcn * 128 + np.arange(128, dtype=np.float64)
        B32[:, N + T + tcn] = (1e-6 * U_DECAY ** (-ts)).astype(np.float32)
    B16 = np.zeros((128, W16), dtype=np.float16)
    B16[:, 0:N] = Dy.reshape(C, 128, D).transpose(2, 0, 1).reshape(128, N)
    B16[:, N:2 * N] = E.reshape(D, C, 128).transpose(2, 1, 0).reshape(128, N)
    # strict-causal mask: mask[sc][i, t] = (sc*128 + i) < t
    s_all = np.arange(T)[:, None]
    t_all = np.arange(T)[None, :]
    m = (s_all < t_all).astype(np.float16)           # [s, t]
    B16[:, 2 * N + TC * 128:2 * N + TC * 128 + TC * T] = (
        m.reshape(TC, 128, T).transpose(1, 0, 2).reshape(128, TC * T))
    B16[:, 2 * N + TC * 128 + TC * T:] = np.eye(128, dtype=np.float16)
    return B32, B16


def _host_prep_core(B32t, B16t, token_emb, tokens_b, T):
    TC = T // 128
    B32 = B32t.copy()
    B16 = B16t.copy()
    V_all = token_emb[tokens_b].astype(np.float32)         # [T, 128]
    B32[:, N:N + T] = V_all.T
    decay = (U_DECAY ** (-np.arange(T, dtype=np.float64))).astype(np.float32)
    Vh_flat = (V_all * decay[:, None]).astype(np.float16)  # [T, 128]
    B16[:, 2 * N:2 * N + TC * 128] = (
        Vh_flat.reshape(TC, 128, 128).transpose(1, 0, 2).reshape(128, TC * 128))
    return dict(B32=B32, B16=B16)


_PROGRAM_CACHE = {}
RUN_KWARGS = {}      # extra kwargs forwarded to run_bass_kernel_spmd
LAST_RESULTS = None  # BassKernelResults of the most recent kernel() call


def _build(T):
    key = T
    if key in _PROGRAM_CACHE:
        return _PROGRAM_CACHE[key]
    TC = T // 128
    W32 = N + T + TC
    W16 = 2 * N + TC * 128 + TC * T + 128
    nc = bacc.Bacc("TRN2")
    ins = {
        "B32": nc.dram_tensor("B32", [128, W32], F32, kind="ExternalInput").ap(),
        "B16": nc.dram_tensor("B16", [128, W16], F16, kind="ExternalInput").ap(),
    }
    outs = {
        "out": nc.dram_tensor("out", [T, D], F32, kind="ExternalOutput").ap(),
    }
    with tile.TileContext(nc) as tc:
        scan_program(tc, outs, ins, T)
    nc.compile()
    _PROGRAM_CACHE[key] = (nc, ins, outs)
    return _PROGRAM_CACHE[key]


def kernel(E, Dx, Dy, token_emb, tokens):
    from concourse.bass_utils import run_bass_kernel_spmd

    E = np.asarray(E, dtype=np.float32)
    Dx = np.asarray(Dx, dtype=np.float32)
    Dy = np.asarray(Dy, dtype=np.float32)
    token_emb = np.asarray(token_emb, dtype=np.float32)
    tokens = np.asarray(tokens)
    B, T = tokens.shape

    nc, ins, outs = _build(T)
    B32t, B16t = _host_prep_shared(E, Dx, Dy, T)
    in_maps = [
        _host_prep_core(B32t, B16t, token_emb, tokens[b], T) for b in range(B)
    ]

    res = run_bass_kernel_spmd(nc, in_maps, core_ids=list(range(B)), **RUN_KWARGS)
    global LAST_RESULTS
    LAST_RESULTS = res
    out = np.stack([r["out"] for r in res.results])  # [B, T, 128]
    return out.astype(np.float32)


# revision 28
# speedup vs baseline: 1.1063x; 1.0189x over previous
"""Trainium2 Bass kernel for nn_BDHGPURefStabilized.

Model (per batch element b, scan over T steps):
    v_t   = token_emb[tok_t]                         # [D]
    xt    = 0.97*x + v_t @ Dx.T                      # [N]
    xt    = xt / (sum|xt| + 1e-6)
    xt    = where(xt > 0.02*max(xt), xt, 0)
    a*    = rho @ xt                                 # fast-weight read [D]
    y     = LN(a*) @ Dy.T                            # [N]
    yt    = relu(y) * relu(xt)
    v*_t  = LN(yt @ E.T)                             # output row [D]
    rho   = 0.97*(rho + v_t (x) xt)                  # rank-1 fast-weight update

Kernel strategy (8 NeuronCores, data-parallel over batch B=8, one batch
element per core, zero collectives):

 - Split the computation into a minimal serial spine and a fully batched
   output chain.  The spine is the only true recurrence: with the
   rescaling w_t = xt_t/0.97 and host-prescaled P~ = (v @ Dx.T)/0.97,
       w_{t+1} = (w_t > 0.02 max(w_t)) * w_t / sum|w_t| + P~_{t+1}
   and the masked-normalized history is x_t = w_{t+1} - P~_{t+1}.
   Per step: two DVE free-axis reductions, two gpsimd partition_all_reduce
   ops (cross-partition max / sum, result broadcast to all partitions),
   and three DVE element-wise ops.  History (x_t and relu(x_t), fp16) is
   written by the otherwise-idle ACT engine off the critical path.
 - The output chain is a pure function of the history, so it runs once,
   batched over all T as large matmuls: G = Xh^T Xh (Gram vs history),
   strict-causal mask, A = G^T @ (0.97^{-s} v_s) with the 0.97^t factor
   folded into a per-row LayerNorm epsilon, then LN, y = LN(A) @ Dy.T,
   yt = relu(y)*relu(x), u = yt @ E.T, out = LN(u) — all LayerNorms
   batched 128 rows/op via bn_stats.  Output-path matmuls run fp16
   (PSUM accumulates fp32); the spine stays fp32.
 - This takes the Tensor-engine sequencer from ~28k instructions (the
   previous per-step formulation) to ~120, and the per-step critical
   path from ~4.5us to ~0.6us.

Output per core: [T, 128] fp32 rows; host stacks [B, T, D].
"""

from contextlib import ExitStack

import numpy as np

import concourse.bass as bass
import concourse.bacc as bacc
import concourse.tile as tile
from concourse import bass_isa, mybir

F32 = mybir.dt.float32
F16 = mybir.dt.float16
AX = mybir.AxisListType
OP = mybir.AluOpType
AF = mybir.ActivationFunctionType
RED = bass_isa.ReduceOp

N, D, V = 2048, 128, 131072
C = N // 128  # 16 column-chunks of n; n = c*128 + j
U_DECAY, X_DECAY, THR = 0.97, 0.97, 0.02

# spine emission variant (selected by TimelineSim sweep)
SPINE = {
    "thr_mode": "post",     # "pre" | "post"
    "pool_order": "max_first",  # "max_first" | "sum_first"
    "red_order": "max_first",   # "max_first" | "sum_first"
    "hist": "bulk",         # "act" | "bulk"
    "keepalive": True,      # per-step PE dummy matmul (p-state warm-keeping)
}


def scan_program(tc, outs, ins, T):
    nc = tc.nc
    ctx = ExitStack()
    TC = T // 128          # t-chunks (2 for T=256)
    W32 = N + T + TC       # packed f32 input width
    W16 = 2 * N + TC * 128 + TC * T + 128  # packed f16 input width

    with ctx:
        wpool = ctx.enter_context(tc.tile_pool(name="weights", bufs=1))
        spool = ctx.enter_context(tc.tile_pool(name="step", bufs=3))
        scal = ctx.enter_context(tc.tile_pool(name="scal", bufs=4))

        B32 = wpool.tile([128, W32], F32, tag="B32")
        B16 = wpool.tile([128, W16], F16, tag="B16")
        nc.sync.dma_start(out=B32, in_=ins["B32"])
        nc.sync.dma_start(out=B16, in_=ins["B16"])
        DxT = B32[:, 0:N]                          # [d, n]/0.97
        Vt = B32[:, N:N + T]                       # [d, t]
        eps2 = B32[:, N + T:N + T + TC]            # LN(A) eps per t-chunk
        DyTr = B16[:, 0:N].rearrange("p (c j) -> p c j", c=C)      # [d,(c,j)]
        ETr = B16[:, N:2 * N].rearrange("p (c j) -> p c j", c=C)   # [j,(c,d)]
        Vh = B16[:, 2 * N:2 * N + TC * 128].rearrange(
            "p (s j) -> p s j", s=TC)                              # [s,(sc,d)]
        mask16 = B16[:, 2 * N + TC * 128:2 * N + TC * 128 + TC * T].rearrange(
            "p (s j) -> p s j", s=TC)                              # [s,(sc,t)]
        idn16 = B16[:, 2 * N + TC * 128 + TC * T:]                 # [128,128] I

        # persistent SBUF state.  Spine state is laid out with one column per
        # step (zero buffer reuse): every spine instruction then carries at
        # most ONE semaphore wait, so bacc's multi-wait splitting never
        # inserts SEQ-blocking EventSemaphore instructions into the loop.
        P_sb = wpool.tile([128, T, C], F32, tag="P_sb")    # P~ = v@Dx.T/0.97
        Xh = wpool.tile([128, C, T], F16, tag="Xh")        # x_t history
        W16h = wpool.tile([128, C, T], F16, tag="W16h")    # relu(x_t) history
        Wsp = wpool.tile([128, T, C], F32, tag="Wsp")      # spine state u_t
        Ysp = wpool.tile([128, T, C], F32, tag="Ysp")      # masked u_t
        red2T = wpool.tile([128, 2, T], F32, tag="red2T")  # partial reduces
        ar2T = wpool.tile([128, 2, T], F32, tag="ar2T")    # all-reduced S, m
        thrT = wpool.tile([128, 1, T], F32, tag="thrT")
        invT = wpool.tile([128, 1, T], F32, tag="invT")
        rall = wpool.tile([128, T], F32, tag="rall")       # 1/S_t (bulk)

        # ---- PE warm-up: ~3us of junk matmuls overlapping the input DMA
        # ramps the tensor engine to full clock before the fp32 P~ matmuls
        # (cold fp32 matmuls run at 2-4x the cycle time).
        warm = wpool.tile([128, 256], F16, tag="warm")
        nc.vector.memset(warm, 0.0)
        # dummy Sqrt+Relu so the ACT function-table loads happen here, in
        # DMA dead time, instead of mid-way through the output chain
        aw = wpool.tile([1, 2], F32, tag="actwarm")
        nc.vector.memset(aw, 1.0)
        nc.scalar.activation(out=aw[:, 1:2], in_=aw[:, 1:2], func=AF.Relu)
        nc.scalar.activation(out=aw[:, 0:1], in_=aw[:, 0:1], func=AF.Sqrt)
        with tc.tile_pool(name="pwarm", bufs=2, space="PSUM") as pwarm:
            for i in range(14):
                w_ps = pwarm.tile([128, 256], F32, tag="w")
                nc.tensor.matmul(
                    w_ps, warm[:, 0:128], warm, start=True, stop=True)

        # ---- P~ = DxT~ @ V (device-side, fp32) ----
        # PSUM evacuation on DVE (not ACT) so the spine's P_sb readers wait
        # on the DVE semaphore only (coalesces with their other DVE waits).
        with tc.tile_pool(name="psetup", bufs=2, space="PSUM") as psetup:
            for c in range(C):
                p_ps = psetup.tile([128, T], F32, tag="pp")
                nc.tensor.matmul(
                    p_ps, DxT[:, c * 128:(c + 1) * 128], Vt,
                    start=True, stop=True,
                )
                nc.vector.tensor_copy(P_sb[:, :, c], p_ps)

        # ---- serial spine ----
        # The 0.02 threshold scale is applied to the PER-PARTITION partial
        # maxima before the cross-partition allreduce (exact: fl(0.02*x) is
        # monotone, so max_p fl(0.02*m_p) == fl(0.02*max_p m_p)), so the
        # Pool allreduce returns thr directly and the mask STT carries a
        # single Pool-sem wait.  Pool stays PartitionAllReduce-only (any
        # standard gpsimd op in the loop would force a Q7 library reload
        # per step).  Histories are normalized in bulk after the loop.
        pkeep = ctx.enter_context(
            tc.tile_pool(name="pkeep", bufs=2, space="PSUM"))
        for t in range(T):
            u = Wsp[:, t, :] if t > 0 else P_sb[:, 0, :]
            pmax = red2T[:, 1, t:t + 1]
            psum = red2T[:, 0, t:t + 1]
            armax = ar2T[:, 1, t:t + 1]
            arsum = ar2T[:, 0, t:t + 1]
            thrp = thrT[:, 0, t:t + 1]
            invs = invT[:, 0, t:t + 1]
            y = Ysp[:, t, :]

            def emit_redmax():
                nc.vector.tensor_reduce(
                    out=pmax, in_=u, axis=AX.X, op=OP.max)
                if SPINE["thr_mode"] == "pre":
                    # pre-scale by 0.02 (exact: fl(0.02*x) is monotone) so
                    # the Pool allreduce returns thr directly
                    nc.vector.tensor_scalar(
                        out=thrp, in0=pmax, scalar1=float(THR),
                        scalar2=None, op0=OP.mult)

            def emit_redsum():
                nc.vector.tensor_reduce(
                    out=psum, in_=u, axis=AX.X, op=OP.add,
                    apply_absolute_value=True)

            def emit_par_max():
                nc.gpsimd.partition_all_reduce(
                    armax, thrp if SPINE["thr_mode"] == "pre" else pmax,
                    128, RED.max)

            def emit_par_sum():
                nc.gpsimd.partition_all_reduce(arsum, psum, 128, RED.add)

            if SPINE["red_order"] == "max_first":
                emit_redmax()
                emit_redsum()
            else:
                emit_redsum()
                emit_redmax()
            if SPINE["pool_order"] == "max_first":
                emit_par_max()
                emit_par_sum()
            else:
                emit_par_sum()
                emit_par_max()
            if SPINE["thr_mode"] == "post":
                nc.vector.tensor_scalar(
                    out=thrp, in0=armax, scalar1=float(THR),
                    scalar2=None, op0=OP.mult)
                thr_ap = thrp
            else:
                thr_ap = armax
            nc.vector.scalar_tensor_tensor(
                out=y, in0=u, scalar=thr_ap, in1=u,
                op0=OP.is_gt, op1=OP.mult)
            nc.vector.reciprocal(out=invs, in_=arsum)
            if t + 1 < T:
                nc.vector.scalar_tensor_tensor(
                    out=Wsp[:, t + 1, :], in0=y, scalar=invs,
                    in1=P_sb[:, t + 1, :], op0=OP.mult, op1=OP.add)
            if SPINE.get("keepalive"):
                # tiny per-step matmul keeps the PE p-state ramp alive so
                # the post-spine batched matmuls start at full clock
                k_ps = pkeep.tile([1, 2], F32, tag="k")
                nc.tensor.matmul(
                    k_ps, DxT[:, 0:1], red2T[:, :, t], start=True, stop=True)
            if SPINE["hist"] == "act":
                nc.scalar.activation(
                    out=Xh[:, :, t], in_=y, func=AF.Copy, scale=invs)
                nc.scalar.activation(
                    out=W16h[:, :, t], in_=y, func=AF.Relu, scale=invs)
        if SPINE["hist"] == "bulk":
            nc.vector.reciprocal(out=rall, in_=ar2T[:, 0, :])
            for c in range(C):
                # split the 16 normalization multiplies between DVE and the
                # otherwise-idle Pool engine (standard-lib ops are fine here,
                # the spine's allreduce stream is over)
                eng = nc.gpsimd if c % 3 == 2 else nc.vector
                eng.tensor_tensor(
                    out=Xh[:, c, :], in0=Ysp[:, :, c], in1=rall, op=OP.mult)
                nc.scalar.activation(
                    out=W16h[:, c, :], in_=Xh[:, c, :], func=AF.Relu)

        # ---- batched output chain ----
        # G[s, t] = x_s . x_t   (strict-causal masked, fp16)
        G16 = wpool.tile([128, TC, T], F16, tag="G16")
        with tc.tile_pool(name="pG", bufs=2, space="PSUM") as pG:
            for sc in range(TC):
                g_ps = pG.tile([128, T], F32, tag="g")
                for c in range(C):
                    nc.tensor.matmul(
                        g_ps, Xh[:, c, sc * 128:(sc + 1) * 128], Xh[:, c, :],
                        start=(c == 0), stop=(c == C - 1),
                    )
                nc.vector.tensor_tensor(
                    out=G16[:, sc, :], in0=g_ps, in1=mask16[:, sc, :],
                    op=OP.mult)

        # A[t, d] = sum_s G[s, t] Vh[s, d];  LnA = rowwise-LN(A, eps_t)
        LnAT = wpool.tile([128, T], F16, tag="LnAT")     # [d, t]
        with tc.tile_pool(name="pA", bufs=2, space="PSUM") as pA, \
                tc.tile_pool(name="pT", bufs=2, space="PSUM") as pT:
            for tcn in range(TC):
                a_ps = pA.tile([128, 128], F32, tag="a")
                for sc in range(TC):
                    nc.tensor.matmul(
                        a_ps, G16[:, sc, tcn * 128:(tcn + 1) * 128],
                        Vh[:, sc, :], start=(sc == 0), stop=(sc == TC - 1),
                    )
                lnA = _layernorm_rows(
                    tc, spool, scal, a_ps, eps2[:, tcn:tcn + 1], F16)
                t_ps = pT.tile([128, 128], F16, tag="t")
                nc.tensor.transpose(t_ps, lnA, idn16)
                nc.scalar.copy(LnAT[:, tcn * 128:(tcn + 1) * 128], t_ps)

        # yT[n, t] = Dy @ LnA^T;  yt = relu(yT) * relu(x)  (fp16)
        yt_sb = wpool.tile([128, C, T], F16, tag="yt")
        with tc.tile_pool(name="pY", bufs=3, space="PSUM") as pY:
            for c in range(C):
                y_ps = pY.tile([128, T], F32, tag="y")
                nc.tensor.matmul(y_ps, DyTr[:, c, :], LnAT,
                                 start=True, stop=True)
                if c % 3 == 0:
                    # DVE: fused relu+mask-multiply straight from PSUM
                    nc.vector.scalar_tensor_tensor(
                        out=yt_sb[:, c, :], in0=y_ps, scalar=0.0,
                        in1=W16h[:, c, :], op0=OP.max, op1=OP.mult)
                else:
                    # ACT evacuates with relu; DVE/Pool multiply fp16
                    ry = spool.tile([128, T], F16, tag="ry")
                    nc.scalar.activation(out=ry, in_=y_ps, func=AF.Relu)
                    eng = nc.vector if c % 3 == 1 else nc.gpsimd
                    eng.tensor_tensor(
                        out=yt_sb[:, c, :], in0=ry, in1=W16h[:, c, :],
                        op=OP.mult)

        # u[t, d] = sum_n yt[n, t] E[d, n];  out = rowwise-LN(u)
        with tc.tile_pool(name="pU", bufs=2, space="PSUM") as pU:
            for tcn in range(TC):
                u_ps = pU.tile([128, 128], F32, tag="u")
                for c in range(C):
                    nc.tensor.matmul(
                        u_ps, yt_sb[:, c, tcn * 128:(tcn + 1) * 128],
                        ETr[:, c, :], start=(c == 0), stop=(c == C - 1),
                    )
                o_sb = _layernorm_rows(tc, spool, scal, u_ps, 1e-6, F32)
                nc.sync.dma_start(
                    out=outs["out"][tcn * 128:(tcn + 1) * 128, :], in_=o_sb)


def _layernorm_rows(tc, spool, scal, rows_ps, eps, out_dtype):
    """Row-wise LayerNorm of a [128, 128] PSUM tile (torch-style: ddof=1,
    eps added to std).  ``eps`` is a float or a [128, 1] AP (per-row).
    Returns a [128, 128] SBUF tile of out_dtype."""
    nc = tc.nc
    stats = scal.tile([128, 6], F32, tag="ln_stats")
    mv = scal.tile([128, 2], F32, tag="ln_mv")
    nc.vector.bn_stats(out=stats, in_=rows_ps)
    nc.vector.bn_aggr(out=mv, in_=stats)
    sd = scal.tile([128, 2], F32, tag="ln_sd")
    nc.scalar.activation(
        out=sd[:, 0:1], in_=mv[:, 1:2], func=AF.Sqrt,
        scale=float(D) / (D - 1))
    if isinstance(eps, float):
        nc.vector.tensor_scalar(
            out=sd[:, 1:2], in0=sd[:, 0:1], scalar1=eps, scalar2=None,
            op0=OP.add)
    else:
        nc.vector.tensor_scalar(
            out=sd[:, 1:2], in0=sd[:, 0:1], scalar1=eps, scalar2=None,
            op0=OP.add)
    rstd = scal.tile([128, 1], F32, tag="ln_rstd")
    nc.vector.reciprocal(out=rstd, in_=sd[:, 1:2])
    out = spool.tile([128, 128], out_dtype, tag=f"ln_out_{out_dtype}")
    nc.vector.tensor_scalar(
        out=out, in0=rows_ps, scalar1=mv[:, 0:1], scalar2=rstd,
        op0=OP.subtract, op1=OP.mult)
    return out


# ----------------------------------------------------------------------------
# host side
# ----------------------------------------------------------------------------

def _host_prep_shared(E, Dx, Dy, T):
    """Packed B32/B16 templates (per-core slots for Vt/Vh left zero)."""
    TC = T // 128
    W32 = N + T + TC
    W16 = 2 * N + TC * 128 + TC * T + 128
    B32 = np.zeros((128, W32), dtype=np.float32)
    B32[:, 0:N] = Dx.T / X_DECAY
    for tcn in range(TC):
        ts = tcn * 128 + np.arange(128, dtype=np.float64)
        B32[:, N + T + tcn] = (1e-6 * U_DECAY ** (-ts)).astype(np.float32)
    B16 = np.zeros((128, W16), dtype=np.float16)
    B16[:, 0:N] = Dy.reshape(C, 128, D).transpose(2, 0, 1).reshape(128, N)
    B16[:, N:2 * N] = E.reshape(D, C, 128).transpose(2, 1, 0).reshape(128, N)
    # strict-causal mask: mask[sc][i, t] = (sc*128 + i) < t
    s_all = np.arange(T)[:, None]
    t_all = np.arange(T)[None, :]
    m = (s_all < t_all).astype(np.float16)           # [s, t]
    B16[:, 2 * N + TC * 128:2 * N + TC * 128 + TC * T] = (
        m.reshape(TC, 128, T).transpose(1, 0, 2).reshape(128, TC * T))
    B16[:, 2 * N + TC * 128 + TC * T:] = np.eye(128, dtype=np.float16)
    return B32, B16


def _host_prep_core(B32t, B16t, token_emb, tokens_b, T):
    TC = T // 128
    B32 = B32t.copy()
    B16 = B16t.copy()
    V_all = token_emb[tokens_b].astype(np.float32)         # [T, 128]
    B32[:, N:N + T] = V_all.T
    decay = (U_DECAY ** (-np.arange(T, dtype=np.float64))).astype(np.float32)
    Vh_flat = (V_all * decay[:, None]).astype(np.float16)  # [T, 128]
    B16[:, 2 * N:2 * N + TC * 128] = (
        Vh_flat.reshape(TC, 128, 128).transpose(1, 0, 2).reshape(128, TC * 128))
    return dict(B32=B32, B16=B16)


_PROGRAM_CACHE = {}
RUN_KWARGS = {}      # extra kwargs forwarded to run_bass_kernel_spmd
LAST_RESULTS = None  # BassKernelResults of the most recent kernel() call


def _build(T):
    key = T
    if key in _PROGRAM_CACHE:
        return _PROGRAM_CACHE[key]
    TC = T // 128
    W32 = N + T + TC
    W16 = 2 * N + TC * 128 + TC * T + 128
    nc = bacc.Bacc("TRN2")
    ins = {
        "B32": nc.dram_tensor("B32", [128, W32], F32, kind="ExternalInput").ap(),
        "B16": nc.dram_tensor("B16", [128, W16], F16, kind="ExternalInput").ap(),
    }
    outs = {
        "out": nc.dram_tensor("out", [T, D], F32, kind="ExternalOutput").ap(),
    }
    with tile.TileContext(nc) as tc:
        scan_program(tc, outs, ins, T)
    nc.compile()
    _PROGRAM_CACHE[key] = (nc, ins, outs)
    return _PROGRAM_CACHE[key]


def kernel(E, Dx, Dy, token_emb, tokens):
    from concourse.bass_utils import run_bass_kernel_spmd

    E = np.asarray(E, dtype=np.float32)
    Dx = np.asarray(Dx, dtype=np.float32)
    Dy = np.asarray(Dy, dtype=np.float32)
    token_emb = np.asarray(token_emb, dtype=np.float32)
    tokens = np.asarray(tokens)
    B, T = tokens.shape

    nc, ins, outs = _build(T)
    B32t, B16t = _host_prep_shared(E, Dx, Dy, T)
    in_maps = [
        _host_prep_core(B32t, B16t, token_emb, tokens[b], T) for b in range(B)
    ]

    res = run_bass_kernel_spmd(nc, in_maps, core_ids=list(range(B)), **RUN_KWARGS)
    global LAST_RESULTS
    LAST_RESULTS = res
    out = np.stack([r["out"] for r in res.results])  # [B, T, 128]
    return out.astype(np.float32)


# revision 35
# speedup vs baseline: 1.1133x; 1.0063x over previous
"""Trainium2 Bass kernel for nn_BDHGPURefStabilized.

Model (per batch element b, scan over T steps):
    v_t   = token_emb[tok_t]                         # [D]
    xt    = 0.97*x + v_t @ Dx.T                      # [N]
    xt    = xt / (sum|xt| + 1e-6)
    xt    = where(xt > 0.02*max(xt), xt, 0)
    a*    = rho @ xt                                 # fast-weight read [D]
    y     = LN(a*) @ Dy.T                            # [N]
    yt    = relu(y) * relu(xt)
    v*_t  = LN(yt @ E.T)                             # output row [D]
    rho   = 0.97*(rho + v_t (x) xt)                  # rank-1 fast-weight update

Kernel strategy (8 NeuronCores, data-parallel over batch B=8, one batch
element per core, zero collectives):

 - Split the computation into a minimal serial spine and a fully batched
   output chain.  The spine is the only true recurrence: with the
   rescaling w_t = xt_t/0.97 and host-prescaled P~ = (v @ Dx.T)/0.97,
       w_{t+1} = (w_t > 0.02 max(w_t)) * w_t / sum|w_t| + P~_{t+1}
   and the masked-normalized history is x_t = w_{t+1} - P~_{t+1}.
   Per step: two DVE free-axis reductions, two gpsimd partition_all_reduce
   ops (cross-partition max / sum, result broadcast to all partitions),
   and three DVE element-wise ops.  History (x_t and relu(x_t), fp16) is
   written by the otherwise-idle ACT engine off the critical path.
 - The output chain is a pure function of the history, so it runs once,
   batched over all T as large matmuls: G = Xh^T Xh (Gram vs history),
   strict-causal mask, A = G^T @ (0.97^{-s} v_s) with the 0.97^t factor
   folded into a per-row LayerNorm epsilon, then LN, y = LN(A) @ Dy.T,
   yt = relu(y)*relu(x), u = yt @ E.T, out = LN(u) — all LayerNorms
   batched 128 rows/op via bn_stats.  Output-path matmuls run fp16
   (PSUM accumulates fp32); the spine stays fp32.
 - This takes the Tensor-engine sequencer from ~28k instructions (the
   previous per-step formulation) to ~120, and the per-step critical
   path from ~4.5us to ~0.6us.

Output per core: [T, 128] fp32 rows; host stacks [B, T, D].
"""

from contextlib import ExitStack

import numpy as np

import concourse.bass as bass
import concourse.bacc as bacc
import concourse.tile as tile
from concourse import bass_isa, mybir

F32 = mybir.dt.float32
F16 = mybir.dt.float16
AX = mybir.AxisListType
OP = mybir.AluOpType
AF = mybir.ActivationFunctionType
RED = bass_isa.ReduceOp

N, D, V = 2048, 128, 131072
C = N // 128  # 16 column-chunks of n; n = c*128 + j
U_DECAY, X_DECAY, THR = 0.97, 0.97, 0.02

# spine emission variant (selected by TimelineSim sweep)
SPINE = {
    "thr_mode": "post",     # "pre" | "post"
    "pool_order": "max_first",  # "max_first" | "sum_first"
    "red_order": "max_first",   # "max_first" | "sum_first"
    "hist": "bulk",         # "act" | "bulk"
    "keepalive": True,      # per-step PE dummy matmul (p-state warm-keeping)
}


def scan_program(tc, outs, ins, T):
    nc = tc.nc
    ctx = ExitStack()
    TC = T // 128          # t-chunks (2 for T=256)
    W32 = N + T + TC       # packed f32 input width
    W16 = 2 * N + TC * 128 + TC * T + 128  # packed f16 input width

    with ctx:
        wpool = ctx.enter_context(tc.tile_pool(name="weights", bufs=1))
        spool = ctx.enter_context(tc.tile_pool(name="step", bufs=3))
        scal = ctx.enter_context(tc.tile_pool(name="scal", bufs=4))

        B32a = wpool.tile([128, T + TC + N // 2], F32, tag="B32a")
        B32b = wpool.tile([128, N // 2], F32, tag="B32b")
        B16 = wpool.tile([128, W16], F16, tag="B16")
        # B32 is split so the first P~ matmuls start after ~half the f32
        # input has landed
        nc.sync.dma_start(out=B32a, in_=ins["B32a"])
        nc.sync.dma_start(out=B32b, in_=ins["B32b"])
        nc.sync.dma_start(out=B16, in_=ins["B16"])
        Vt = B32a[:, 0:T]                          # [d, t]
        eps2 = B32a[:, T:T + TC]                   # LN(A) eps per t-chunk

        def DxTc(c):
            # [d, 128] chunk c of Dx.T/0.97
            if c < C // 2:
                return B32a[:, T + TC + c * 128:T + TC + (c + 1) * 128]
            return B32b[:, (c - C // 2) * 128:(c - C // 2 + 1) * 128]
        DyTr = B16[:, 0:N].rearrange("p (c j) -> p c j", c=C)      # [d,(c,j)]
        ETr = B16[:, N:2 * N].rearrange("p (c j) -> p c j", c=C)   # [j,(c,d)]
        Vh = B16[:, 2 * N:2 * N + TC * 128].rearrange(
            "p (s j) -> p s j", s=TC)                              # [s,(sc,d)]
        mask16 = B16[:, 2 * N + TC * 128:2 * N + TC * 128 + TC * T].rearrange(
            "p (s j) -> p s j", s=TC)                              # [s,(sc,t)]
        idn16 = B16[:, 2 * N + TC * 128 + TC * T:]                 # [128,128] I

        # persistent SBUF state.  Spine state is laid out with one column per
        # step (zero buffer reuse): every spine instruction then carries at
        # most ONE semaphore wait, so bacc's multi-wait splitting never
        # inserts SEQ-blocking EventSemaphore instructions into the loop.
        P_sb = wpool.tile([128, T, C], F32, tag="P_sb")    # P~ = v@Dx.T/0.97
        Xh = wpool.tile([128, C, T], F16, tag="Xh")        # x_t history
        W16h = wpool.tile([128, C, T], F16, tag="W16h")    # relu(x_t) history
        Wsp = wpool.tile([128, T, C], F32, tag="Wsp")      # spine state u_t
        Ysp = wpool.tile([128, T, C], F32, tag="Ysp")      # masked u_t
        red2T = wpool.tile([128, 2, T], F32, tag="red2T")  # partial reduces
        ar2T = wpool.tile([128, 2, T], F32, tag="ar2T")    # all-reduced S, m
        thrT = wpool.tile([128, 1, T], F32, tag="thrT")
        invT = wpool.tile([128, 1, T], F32, tag="invT")
        rall = wpool.tile([128, T], F32, tag="rall")       # 1/S_t (bulk)

        # ---- PE warm-up: ~3us of junk matmuls overlapping the input DMA
        # ramps the tensor engine to full clock before the fp32 P~ matmuls
        # (cold fp32 matmuls run at 2-4x the cycle time).
        warm = wpool.tile([128, 256], F16, tag="warm")
        nc.vector.memset(warm, 0.0)
        # dummy Sqrt+Relu so the ACT function-table loads happen here, in
        # DMA dead time, instead of mid-way through the output chain
        aw = wpool.tile([1, 2], F32, tag="actwarm")
        nc.vector.memset(aw, 1.0)
        nc.scalar.activation(out=aw[:, 1:2], in_=aw[:, 1:2], func=AF.Relu)
        nc.scalar.activation(out=aw[:, 0:1], in_=aw[:, 0:1], func=AF.Sqrt)
        with tc.tile_pool(name="pwarm", bufs=2, space="PSUM") as pwarm:
            for i in range(14):
                w_ps = pwarm.tile([128, 256], F32, tag="w")
                nc.tensor.matmul(
                    w_ps, warm[:, 0:128], warm, start=True, stop=True)

        # ---- P~ = DxT~ @ V (device-side, fp32) ----
        # PSUM evacuation on DVE (not ACT) so the spine's P_sb readers wait
        # on the DVE semaphore only (coalesces with their other DVE waits).
        with tc.tile_pool(name="psetup", bufs=2, space="PSUM") as psetup:
            for c in range(C):
                p_ps = psetup.tile([128, T], F32, tag="pp")
                nc.tensor.matmul(
                    p_ps, DxTc(c), Vt,
                    start=True, stop=True,
                )
                nc.vector.tensor_copy(P_sb[:, :, c], p_ps)

        # ---- serial spine ----
        # The 0.02 threshold scale is applied to the PER-PARTITION partial
        # maxima before the cross-partition allreduce (exact: fl(0.02*x) is
        # monotone, so max_p fl(0.02*m_p) == fl(0.02*max_p m_p)), so the
        # Pool allreduce returns thr directly and the mask STT carries a
        # single Pool-sem wait.  Pool stays PartitionAllReduce-only (any
        # standard gpsimd op in the loop would force a Q7 library reload
        # per step).  Histories are normalized in bulk after the loop.
        pkeep = ctx.enter_context(
            tc.tile_pool(name="pkeep", bufs=2, space="PSUM"))
        for t in range(T):
            u = Wsp[:, t, :] if t > 0 else P_sb[:, 0, :]
            pmax = red2T[:, 1, t:t + 1]
            psum = red2T[:, 0, t:t + 1]
            armax = ar2T[:, 1, t:t + 1]
            arsum = ar2T[:, 0, t:t + 1]
            thrp = thrT[:, 0, t:t + 1]
            invs = invT[:, 0, t:t + 1]
            y = Ysp[:, t, :]

            def emit_redmax():
                nc.vector.tensor_reduce(
                    out=pmax, in_=u, axis=AX.X, op=OP.max)
                if SPINE["thr_mode"] == "pre":
                    # pre-scale by 0.02 (exact: fl(0.02*x) is monotone) so
                    # the Pool allreduce returns thr directly
                    nc.vector.tensor_scalar(
                        out=thrp, in0=pmax, scalar1=float(THR),
                        scalar2=None, op0=OP.mult)

            def emit_redsum():
                nc.vector.tensor_reduce(
                    out=psum, in_=u, axis=AX.X, op=OP.add,
                    apply_absolute_value=True)

            def emit_par_max():
                nc.gpsimd.partition_all_reduce(
                    armax, thrp if SPINE["thr_mode"] == "pre" else pmax,
                    128, RED.max)

            def emit_par_sum():
                nc.gpsimd.partition_all_reduce(arsum, psum, 128, RED.add)

            if SPINE["red_order"] == "max_first":
                emit_redmax()
                emit_redsum()
            else:
                emit_redsum()
                emit_redmax()
            if SPINE["pool_order"] == "max_first":
                emit_par_max()
                emit_par_sum()
            else:
                emit_par_sum()
                emit_par_max()
            if SPINE["thr_mode"] == "post":
                nc.vector.tensor_scalar(
                    out=thrp, in0=armax, scalar1=float(THR),
                    scalar2=None, op0=OP.mult)
                thr_ap = thrp
            else:
                thr_ap = armax
            nc.vector.scalar_tensor_tensor(
                out=y, in0=u, scalar=thr_ap, in1=u,
                op0=OP.is_gt, op1=OP.mult)
            nc.vector.reciprocal(out=invs, in_=arsum)
            if t + 1 < T:
                nc.vector.scalar_tensor_tensor(
                    out=Wsp[:, t + 1, :], in0=y, scalar=invs,
                    in1=P_sb[:, t + 1, :], op0=OP.mult, op1=OP.add)
            if SPINE.get("keepalive"):
                # tiny per-step matmul keeps the PE p-state ramp alive so
                # the post-spine batched matmuls start at full clock
                k_ps = pkeep.tile([1, 2], F32, tag="k")
                nc.tensor.matmul(
                    k_ps, DxTc(0)[:, 0:1], red2T[:, :, t],
                    start=True, stop=True)
            if SPINE["hist"] == "act":
                nc.scalar.activation(
                    out=Xh[:, :, t], in_=y, func=AF.Copy, scale=invs)
                nc.scalar.activation(
                    out=W16h[:, :, t], in_=y, func=AF.Relu, scale=invs)
        if SPINE["hist"] == "bulk":
            nc.vector.reciprocal(out=rall, in_=ar2T[:, 0, :])
            for c in range(C):
                # split the 16 normalization multiplies between DVE and the
                # otherwise-idle Pool engine (standard-lib ops are fine here,
                # the spine's allreduce stream is over)
                eng = nc.gpsimd if c % 3 == 2 else nc.vector
                eng.tensor_tensor(
                    out=Xh[:, c, :], in0=Ysp[:, :, c], in1=rall, op=OP.mult)
                nc.scalar.activation(
                    out=W16h[:, c, :], in_=Xh[:, c, :], func=AF.Relu)

        # ---- batched output chain ----
        # G[s, t] = x_s . x_t   (strict-causal masked, fp16)
        G16 = wpool.tile([128, TC, T], F16, tag="G16")
        with tc.tile_pool(name="pG", bufs=2, space="PSUM") as pG:
            for sc in range(TC):
                g_ps = pG.tile([128, T], F32, tag="g")
                for c in range(C):
                    nc.tensor.matmul(
                        g_ps, Xh[:, c, sc * 128:(sc + 1) * 128], Xh[:, c, :],
                        start=(c == 0), stop=(c == C - 1),
                    )
                nc.vector.tensor_tensor(
                    out=G16[:, sc, :], in0=g_ps, in1=mask16[:, sc, :],
                    op=OP.mult)

        # A[t, d] = sum_s G[s, t] Vh[s, d];  LnA = rowwise-LN(A, eps_t)
        LnAT = wpool.tile([128, T], F16, tag="LnAT")     # [d, t]
        with tc.tile_pool(name="pA", bufs=2, space="PSUM") as pA, \
                tc.tile_pool(name="pT", bufs=2, space="PSUM") as pT:
            for tcn in range(TC):
                a_ps = pA.tile([128, 128], F32, tag="a")
                for sc in range(TC):
                    nc.tensor.matmul(
                        a_ps, G16[:, sc, tcn * 128:(tcn + 1) * 128],
                        Vh[:, sc, :], start=(sc == 0), stop=(sc == TC - 1),
                    )
                lnA = _layernorm_rows(
                    tc, spool, scal, a_ps, eps2[:, tcn:tcn + 1], F16)
                t_ps = pT.tile([128, 128], F16, tag="t")
                nc.tensor.transpose(t_ps, lnA, idn16)
                nc.scalar.copy(LnAT[:, tcn * 128:(tcn + 1) * 128], t_ps)

        # yT[n, t] = Dy @ LnA^T;  yt = relu(yT) * relu(x)  (fp16)
        yt_sb = wpool.tile([128, C, T], F16, tag="yt")
        with tc.tile_pool(name="pY", bufs=3, space="PSUM") as pY:
            for c in range(C):
                y_ps = pY.tile([128, T], F32, tag="y")
                nc.tensor.matmul(y_ps, DyTr[:, c, :], LnAT,
                                 start=True, stop=True)
                if c % 3 == 0:
                    # DVE: fused relu+mask-multiply straight from PSUM
                    nc.vector.scalar_tensor_tensor(
                        out=yt_sb[:, c, :], in0=y_ps, scalar=0.0,
                        in1=W16h[:, c, :], op0=OP.max, op1=OP.mult)
                else:
                    # ACT evacuates with relu; DVE/Pool multiply fp16
                    ry = spool.tile([128, T], F16, tag="ry")
                    nc.scalar.activation(out=ry, in_=y_ps, func=AF.Relu)
                    eng = nc.vector if c % 3 == 1 else nc.gpsimd
                    eng.tensor_tensor(
                        out=yt_sb[:, c, :], in0=ry, in1=W16h[:, c, :],
                        op=OP.mult)

        # u[t, d] = sum_n yt[n, t] E[d, n];  out = rowwise-LN(u)
        with tc.tile_pool(name="pU", bufs=2, space="PSUM") as pU:
            for tcn in range(TC):
                u_ps = pU.tile([128, 128], F32, tag="u")
                for c in range(C):
                    nc.tensor.matmul(
                        u_ps, yt_sb[:, c, tcn * 128:(tcn + 1) * 128],
                        ETr[:, c, :], start=(c == 0), stop=(c == C - 1),
                    )
                o_sb = _layernorm_rows(tc, spool, scal, u_ps, 1e-6, F32)
                nc.sync.dma_start(
                    out=outs["out"][tcn * 128:(tcn + 1) * 128, :], in_=o_sb)


def _layernorm_rows(tc, spool, scal, rows_ps, eps, out_dtype):
    """Row-wise LayerNorm of a [128, 128] PSUM tile (torch-style: ddof=1,
    eps added to std).  ``eps`` is a float or a [128, 1] AP (per-row).
    Returns a [128, 128] SBUF tile of out_dtype."""
    nc = tc.nc
    stats = scal.tile([128, 6], F32, tag="ln_stats")
    mv = scal.tile([128, 2], F32, tag="ln_mv")
    nc.vector.bn_stats(out=stats, in_=rows_ps)
    nc.vector.bn_aggr(out=mv, in_=stats)
    sd = scal.tile([128, 2], F32, tag="ln_sd")
    nc.scalar.activation(
        out=sd[:, 0:1], in_=mv[:, 1:2], func=AF.Sqrt,
        scale=float(D) / (D - 1))
    if isinstance(eps, float):
        nc.vector.tensor_scalar(
            out=sd[:, 1:2], in0=sd[:, 0:1], scalar1=eps, scalar2=None,
            op0=OP.add)
    else:
        nc.vector.tensor_scalar(
            out=sd[:, 1:2], in0=sd[:, 0:1], scalar1=eps, scalar2=None,
            op0=OP.add)
    rstd = scal.tile([128, 1], F32, tag="ln_rstd")
    nc.vector.reciprocal(out=rstd, in_=sd[:, 1:2])
    out = spool.tile([128, 128], out_dtype, tag=f"ln_out_{out_dtype}")
    nc.vector.tensor_scalar(
        out=out, in0=rows_ps, scalar1=mv[:, 0:1], scalar2=rstd,
        op0=OP.subtract, op1=OP.mult)
    return out


# ----------------------------------------------------------------------------
# host side
# ----------------------------------------------------------------------------

def _host_prep_shared(E, Dx, Dy, T):
    """Packed B32a/B32b/B16 templates (per-core slots for Vt/Vh left zero)."""
    TC = T // 128
    W16 = 2 * N + TC * 128 + TC * T + 128
    DxT = (Dx.T / X_DECAY).astype(np.float32)
    B32a = np.zeros((128, T + TC + N // 2), dtype=np.float32)
    for tcn in range(TC):
        ts = tcn * 128 + np.arange(128, dtype=np.float64)
        B32a[:, T + tcn] = (1e-6 * U_DECAY ** (-ts)).astype(np.float32)
    B32a[:, T + TC:] = DxT[:, 0:N // 2]
    B32b = np.ascontiguousarray(DxT[:, N // 2:])
    B16 = np.zeros((128, W16), dtype=np.float16)
    B16[:, 0:N] = Dy.reshape(C, 128, D).transpose(2, 0, 1).reshape(128, N)
    B16[:, N:2 * N] = E.reshape(D, C, 128).transpose(2, 1, 0).reshape(128, N)
    # strict-causal mask: mask[sc][i, t] = (sc*128 + i) < t
    s_all = np.arange(T)[:, None]
    t_all = np.arange(T)[None, :]
    m = (s_all < t_all).astype(np.float16)           # [s, t]
    B16[:, 2 * N + TC * 128:2 * N + TC * 128 + TC * T] = (
        m.reshape(TC, 128, T).transpose(1, 0, 2).reshape(128, TC * T))
    B16[:, 2 * N + TC * 128 + TC * T:] = np.eye(128, dtype=np.float16)
    return B32a, B32b, B16


def _host_prep_core(B32at, B32bt, B16t, token_emb, tokens_b, T):
    TC = T // 128
    B32a = B32at.copy()
    B16 = B16t.copy()
    V_all = token_emb[tokens_b].astype(np.float32)         # [T, 128]
    B32a[:, 0:T] = V_all.T
    decay = (U_DECAY ** (-np.arange(T, dtype=np.float64))).astype(np.float32)
    Vh_flat = (V_all * decay[:, None]).astype(np.float16)  # [T, 128]
    B16[:, 2 * N:2 * N + TC * 128] = (
        Vh_flat.reshape(TC, 128, 128).transpose(1, 0, 2).reshape(128, TC * 128))
    return dict(B32a=B32a, B32b=B32bt, B16=B16)


_PROGRAM_CACHE = {}
RUN_KWARGS = {}      # extra kwargs forwarded to run_bass_kernel_spmd
LAST_RESULTS = None  # BassKernelResults of the most recent kernel() call


def _build(T):
    key = T
    if key in _PROGRAM_CACHE:
        return _PROGRAM_CACHE[key]
    TC = T // 128
    W32 = N + T + TC
    W16 = 2 * N + TC * 128 + TC * T + 128
    nc = bacc.Bacc("TRN2")
    ins = {
        "B32a": nc.dram_tensor(
            "B32a", [128, T + TC + N // 2], F32, kind="ExternalInput").ap(),
        "B32b": nc.dram_tensor(
            "B32b", [128, N // 2], F32, kind="ExternalInput").ap(),
        "B16": nc.dram_tensor("B16", [128, W16], F16, kind="ExternalInput").ap(),
    }
    outs = {
        "out": nc.dram_tensor("out", [T, D], F32, kind="ExternalOutput").ap(),
    }
    with tile.TileContext(nc) as tc:
        scan_program(tc, outs, ins, T)
    nc.compile()
    _PROGRAM_CACHE[key] = (nc, ins, outs)
    return _PROGRAM_CACHE[key]


def kernel(E, Dx, Dy, token_emb, tokens):
    from concourse.bass_utils import run_bass_kernel_spmd

    E = np.asarray(E, dtype=np.float32)
    Dx = np.asarray(Dx, dtype=np.float32)
    Dy = np.asarray(Dy, dtype=np.float32)
    token_emb = np.asarray(token_emb, dtype=np.float32)
    tokens = np.asarray(tokens)
    B, T = tokens.shape

    nc, ins, outs = _build(T)
    B32at, B32bt, B16t = _host_prep_shared(E, Dx, Dy, T)
    in_maps = [
        _host_prep_core(B32at, B32bt, B16t, token_emb, tokens[b], T)
        for b in range(B)
    ]

    res = run_bass_kernel_spmd(nc, in_maps, core_ids=list(range(B)), **RUN_KWARGS)
    global LAST_RESULTS
    LAST_RESULTS = res
    out = np.stack([r["out"] for r in res.results])  # [B, T, 128]
    return out.astype(np.float32)


# revision 38
# speedup vs baseline: 1.1234x; 1.0091x over previous
"""Trainium2 Bass kernel for nn_BDHGPURefStabilized.

Model (per batch element b, scan over T steps):
    v_t   = token_emb[tok_t]                         # [D]
    xt    = 0.97*x + v_t @ Dx.T                      # [N]
    xt    = xt / (sum|xt| + 1e-6)
    xt    = where(xt > 0.02*max(xt), xt, 0)
    a*    = rho @ xt                                 # fast-weight read [D]
    y     = LN(a*) @ Dy.T                            # [N]
    yt    = relu(y) * relu(xt)
    v*_t  = LN(yt @ E.T)                             # output row [D]
    rho   = 0.97*(rho + v_t (x) xt)                  # rank-1 fast-weight update

Kernel strategy (8 NeuronCores, data-parallel over batch B=8, one batch
element per core, zero collectives):

 - Split the computation into a minimal serial spine and a fully batched
   output chain.  The spine is the only true recurrence: with the
   rescaling w_t = xt_t/0.97 and host-prescaled P~ = (v @ Dx.T)/0.97,
       w_{t+1} = (w_t > 0.02 max(w_t)) * w_t / sum|w_t| + P~_{t+1}
   and the masked-normalized history is x_t = w_{t+1} - P~_{t+1}.
   Per step: two DVE free-axis reductions, two gpsimd partition_all_reduce
   ops (cross-partition max / sum, result broadcast to all partitions),
   and three DVE element-wise ops.  History (x_t and relu(x_t), fp16) is
   written by the otherwise-idle ACT engine off the critical path.
 - The output chain is a pure function of the history, so it runs once,
   batched over all T as large matmuls: G = Xh^T Xh (Gram vs history),
   strict-causal mask, A = G^T @ (0.97^{-s} v_s) with the 0.97^t factor
   folded into a per-row LayerNorm epsilon, then LN, y = LN(A) @ Dy.T,
   yt = relu(y)*relu(x), u = yt @ E.T, out = LN(u) — all LayerNorms
   batched 128 rows/op via bn_stats.  Output-path matmuls run fp16
   (PSUM accumulates fp32); the spine stays fp32.
 - This takes the Tensor-engine sequencer from ~28k instructions (the
   previous per-step formulation) to ~120, and the per-step critical
   path from ~4.5us to ~0.6us.

Output per core: [T, 128] fp32 rows; host stacks [B, T, D].
"""

from contextlib import ExitStack

import numpy as np

import concourse.bass as bass
import concourse.bacc as bacc
import concourse.tile as tile
from concourse import bass_isa, mybir

F32 = mybir.dt.float32
F32R = mybir.dt.float32r
F16 = mybir.dt.float16
AX = mybir.AxisListType
OP = mybir.AluOpType
AF = mybir.ActivationFunctionType
RED = bass_isa.ReduceOp

N, D, V = 2048, 128, 131072
C = N // 128  # 16 column-chunks of n; n = c*128 + j
U_DECAY, X_DECAY, THR = 0.97, 0.97, 0.02

# spine emission variant (selected by TimelineSim sweep)
SPINE = {
    "thr_mode": "post",     # "pre" | "post"
    "pool_order": "max_first",  # "max_first" | "sum_first"
    "red_order": "max_first",   # "max_first" | "sum_first"
    "hist": "bulk",         # "act" | "bulk"
    "keepalive": True,      # per-step PE dummy matmul (p-state warm-keeping)
}


def scan_program(tc, outs, ins, T):
    nc = tc.nc
    ctx = ExitStack()
    TC = T // 128          # t-chunks (2 for T=256)
    W32 = N + T + TC       # packed f32 input width
    W16 = 2 * N + TC * 128 + TC * T + 128  # packed f16 input width

    with ctx:
        wpool = ctx.enter_context(tc.tile_pool(name="weights", bufs=1))
        spool = ctx.enter_context(tc.tile_pool(name="step", bufs=3))
        scal = ctx.enter_context(tc.tile_pool(name="scal", bufs=4))

        B32a = wpool.tile([128, T + TC + N // 2], F32R, tag="B32a")
        B32b = wpool.tile([128, N // 2], F32R, tag="B32b")
        B16 = wpool.tile([128, W16], F16, tag="B16")
        EPS = wpool.tile([128, 2], F32, tag="EPS")
        # B32 is split so the first P~ matmuls start after ~half the f32
        # input has landed
        nc.sync.dma_start(out=B32a, in_=ins["B32a"])
        nc.sync.dma_start(out=EPS, in_=ins["EPS"])
        nc.sync.dma_start(out=B32b, in_=ins["B32b"])
        nc.sync.dma_start(out=B16, in_=ins["B16"])
        Vt = B32a[:, 0:T]                          # [d, t]
        eps2 = EPS[:, 0:TC]                        # LN(A) eps per t-chunk

        def DxTc(c):
            # [d, 128] chunk c of Dx.T/0.97
            if c < C // 2:
                return B32a[:, T + TC + c * 128:T + TC + (c + 1) * 128]
            return B32b[:, (c - C // 2) * 128:(c - C // 2 + 1) * 128]
        DyTr = B16[:, 0:N].rearrange("p (c j) -> p c j", c=C)      # [d,(c,j)]
        ETr = B16[:, N:2 * N].rearrange("p (c j) -> p c j", c=C)   # [j,(c,d)]
        Vh = B16[:, 2 * N:2 * N + TC * 128].rearrange(
            "p (s j) -> p s j", s=TC)                              # [s,(sc,d)]
        mask16 = B16[:, 2 * N + TC * 128:2 * N + TC * 128 + TC * T].rearrange(
            "p (s j) -> p s j", s=TC)                              # [s,(sc,t)]
        idn16 = B16[:, 2 * N + TC * 128 + TC * T:]                 # [128,128] I

        # persistent SBUF state.  Spine state is laid out with one column per
        # step (zero buffer reuse): every spine instruction then carries at
        # most ONE semaphore wait, so bacc's multi-wait splitting never
        # inserts SEQ-blocking EventSemaphore instructions into the loop.
        P_sb = wpool.tile([128, T, C], F32, tag="P_sb")    # P~ = v@Dx.T/0.97
        Xh = wpool.tile([128, C, T], F16, tag="Xh")        # x_t history
        W16h = wpool.tile([128, C, T], F16, tag="W16h")    # relu(x_t) history
        Wsp = wpool.tile([128, T, C], F32, tag="Wsp")      # spine state u_t
        Ysp = wpool.tile([128, T, C], F32, tag="Ysp")      # masked u_t
        red2T = wpool.tile([128, 2, T], F32, tag="red2T")  # partial reduces
        ar2T = wpool.tile([128, 2, T], F32, tag="ar2T")    # all-reduced S, m
        thrT = wpool.tile([128, 1, T], F32, tag="thrT")
        invT = wpool.tile([128, 1, T], F32, tag="invT")
        rall = wpool.tile([128, T], F32, tag="rall")       # 1/S_t (bulk)

        # ---- PE warm-up: ~3us of junk matmuls overlapping the input DMA
        # ramps the tensor engine to full clock before the fp32 P~ matmuls
        # (cold fp32 matmuls run at 2-4x the cycle time).
        warm = wpool.tile([128, 256], F16, tag="warm")
        nc.vector.memset(warm, 0.0)
        # dummy Sqrt+Relu so the ACT function-table loads happen here, in
        # DMA dead time, instead of mid-way through the output chain
        aw = wpool.tile([1, 2], F32, tag="actwarm")
        nc.vector.memset(aw, 1.0)
        nc.scalar.activation(out=aw[:, 1:2], in_=aw[:, 1:2], func=AF.Relu)
        nc.scalar.activation(out=aw[:, 0:1], in_=aw[:, 0:1], func=AF.Sqrt)
        with tc.tile_pool(name="pwarm", bufs=2, space="PSUM") as pwarm:
            for i in range(14):
                w_ps = pwarm.tile([128, 256], F32, tag="w")
                nc.tensor.matmul(
                    w_ps, warm[:, 0:128], warm, start=True, stop=True)

        # ---- P~ = DxT~ @ V (device-side, fp32) ----
        # PSUM evacuation on DVE (not ACT) so the spine's P_sb readers wait
        # on the DVE semaphore only (coalesces with their other DVE waits).
        with tc.tile_pool(name="psetup", bufs=2, space="PSUM") as psetup:
            for c in range(C):
                p_ps = psetup.tile([128, T], F32, tag="pp")
                nc.tensor.matmul(
                    p_ps, DxTc(c), Vt,
                    start=True, stop=True,
                )
                nc.vector.tensor_copy(P_sb[:, :, c], p_ps)

        # ---- serial spine ----
        # The 0.02 threshold scale is applied to the PER-PARTITION partial
        # maxima before the cross-partition allreduce (exact: fl(0.02*x) is
        # monotone, so max_p fl(0.02*m_p) == fl(0.02*max_p m_p)), so the
        # Pool allreduce returns thr directly and the mask STT carries a
        # single Pool-sem wait.  Pool stays PartitionAllReduce-only (any
        # standard gpsimd op in the loop would force a Q7 library reload
        # per step).  Histories are normalized in bulk after the loop.
        pkeep = ctx.enter_context(
            tc.tile_pool(name="pkeep", bufs=2, space="PSUM"))
        for t in range(T):
            u = Wsp[:, t, :] if t > 0 else P_sb[:, 0, :]
            pmax = red2T[:, 1, t:t + 1]
            psum = red2T[:, 0, t:t + 1]
            armax = ar2T[:, 1, t:t + 1]
            arsum = ar2T[:, 0, t:t + 1]
            thrp = thrT[:, 0, t:t + 1]
            invs = invT[:, 0, t:t + 1]
            y = Ysp[:, t, :]

            def emit_redmax():
                nc.vector.tensor_reduce(
                    out=pmax, in_=u, axis=AX.X, op=OP.max)
                if SPINE["thr_mode"] == "pre":
                    # pre-scale by 0.02 (exact: fl(0.02*x) is monotone) so
                    # the Pool allreduce returns thr directly
                    nc.vector.tensor_scalar(
                        out=thrp, in0=pmax, scalar1=float(THR),
                        scalar2=None, op0=OP.mult)

            def emit_redsum():
                nc.vector.tensor_reduce(
                    out=psum, in_=u, axis=AX.X, op=OP.add,
                    apply_absolute_value=True)

            def emit_par_max():
                nc.gpsimd.partition_all_reduce(
                    armax, thrp if SPINE["thr_mode"] == "pre" else pmax,
                    128, RED.max)

            def emit_par_sum():
                nc.gpsimd.partition_all_reduce(arsum, psum, 128, RED.add)

            if SPINE["red_order"] == "max_first":
                emit_redmax()
                emit_redsum()
            else:
                emit_redsum()
                emit_redmax()
            if SPINE["pool_order"] == "max_first":
                emit_par_max()
                emit_par_sum()
            else:
                emit_par_sum()
                emit_par_max()
            if SPINE["thr_mode"] == "post":
                nc.vector.tensor_scalar(
                    out=thrp, in0=armax, scalar1=float(THR),
                    scalar2=None, op0=OP.mult)
                thr_ap = thrp
            else:
                thr_ap = armax
            nc.vector.scalar_tensor_tensor(
                out=y, in0=u, scalar=thr_ap, in1=u,
                op0=OP.is_gt, op1=OP.mult)
            nc.vector.reciprocal(out=invs, in_=arsum)
            if t + 1 < T:
                nc.vector.scalar_tensor_tensor(
                    out=Wsp[:, t + 1, :], in0=y, scalar=invs,
                    in1=P_sb[:, t + 1, :], op0=OP.mult, op1=OP.add)
            if SPINE.get("keepalive"):
                # tiny per-step matmul keeps the PE p-state ramp alive so
                # the post-spine batched matmuls start at full clock
                k_ps = pkeep.tile([1, 2], F32, tag="k")
                nc.tensor.matmul(
                    k_ps, red2T[:, 0, t:t + 1], red2T[:, :, t],
                    start=True, stop=True)
            if SPINE["hist"] == "act":
                nc.scalar.activation(
                    out=Xh[:, :, t], in_=y, func=AF.Copy, scale=invs)
                nc.scalar.activation(
                    out=W16h[:, :, t], in_=y, func=AF.Relu, scale=invs)
        if SPINE["hist"] == "bulk":
            nc.vector.reciprocal(out=rall, in_=ar2T[:, 0, :])
            for c in range(C):
                # split the 16 normalization multiplies between DVE and the
                # otherwise-idle Pool engine (standard-lib ops are fine here,
                # the spine's allreduce stream is over)
                eng = nc.gpsimd if c % 3 == 2 else nc.vector
                eng.tensor_tensor(
                    out=Xh[:, c, :], in0=Ysp[:, :, c], in1=rall, op=OP.mult)
                nc.scalar.activation(
                    out=W16h[:, c, :], in_=Xh[:, c, :], func=AF.Relu)

        # ---- batched output chain ----
        # G[s, t] = x_s . x_t   (strict-causal masked, fp16)
        G16 = wpool.tile([128, TC, T], F16, tag="G16")
        with tc.tile_pool(name="pG", bufs=2, space="PSUM") as pG:
            for sc in range(TC):
                g_ps = pG.tile([128, T], F32, tag="g")
                for c in range(C):
                    nc.tensor.matmul(
                        g_ps, Xh[:, c, sc * 128:(sc + 1) * 128], Xh[:, c, :],
                        start=(c == 0), stop=(c == C - 1),
                    )
                nc.vector.tensor_tensor(
                    out=G16[:, sc, :], in0=g_ps, in1=mask16[:, sc, :],
                    op=OP.mult)

        # A[t, d] = sum_s G[s, t] Vh[s, d];  LnA = rowwise-LN(A, eps_t)
        LnAT = wpool.tile([128, T], F16, tag="LnAT")     # [d, t]
        with tc.tile_pool(name="pA", bufs=2, space="PSUM") as pA, \
                tc.tile_pool(name="pT", bufs=2, space="PSUM") as pT:
            for tcn in range(TC):
                a_ps = pA.tile([128, 128], F32, tag="a")
                for sc in range(TC):
                    nc.tensor.matmul(
                        a_ps, G16[:, sc, tcn * 128:(tcn + 1) * 128],
                        Vh[:, sc, :], start=(sc == 0), stop=(sc == TC - 1),
                    )
                lnA = _layernorm_rows(
                    tc, spool, scal, a_ps, eps2[:, tcn:tcn + 1], F16)
                t_ps = pT.tile([128, 128], F16, tag="t")
                nc.tensor.transpose(t_ps, lnA, idn16)
                nc.scalar.copy(LnAT[:, tcn * 128:(tcn + 1) * 128], t_ps)

        # yT[n, t] = Dy @ LnA^T;  yt = relu(yT) * relu(x)  (fp16)
        yt_sb = wpool.tile([128, C, T], F16, tag="yt")
        with tc.tile_pool(name="pY", bufs=3, space="PSUM") as pY:
            for c in range(C):
                y_ps = pY.tile([128, T], F32, tag="y")
                nc.tensor.matmul(y_ps, DyTr[:, c, :], LnAT,
                                 start=True, stop=True)
                if c % 3 == 0:
                    # DVE: fused relu+mask-multiply straight from PSUM
                    nc.vector.scalar_tensor_tensor(
                        out=yt_sb[:, c, :], in0=y_ps, scalar=0.0,
                        in1=W16h[:, c, :], op0=OP.max, op1=OP.mult)
                else:
                    # ACT evacuates with relu; DVE/Pool multiply fp16
                    ry = spool.tile([128, T], F16, tag="ry")
                    nc.scalar.activation(out=ry, in_=y_ps, func=AF.Relu)
                    eng = nc.vector if c % 3 == 1 else nc.gpsimd
                    eng.tensor_tensor(
                        out=yt_sb[:, c, :], in0=ry, in1=W16h[:, c, :],
                        op=OP.mult)

        # u[t, d] = sum_n yt[n, t] E[d, n];  out = rowwise-LN(u)
        with tc.tile_pool(name="pU", bufs=2, space="PSUM") as pU:
            for tcn in range(TC):
                u_ps = pU.tile([128, 128], F32, tag="u")
                for c in range(C):
                    nc.tensor.matmul(
                        u_ps, yt_sb[:, c, tcn * 128:(tcn + 1) * 128],
                        ETr[:, c, :], start=(c == 0), stop=(c == C - 1),
                    )
                o_sb = _layernorm_rows(tc, spool, scal, u_ps, 1e-6, F32)
                nc.sync.dma_start(
                    out=outs["out"][tcn * 128:(tcn + 1) * 128, :], in_=o_sb)


def _layernorm_rows(tc, spool, scal, rows_ps, eps, out_dtype):
    """Row-wise LayerNorm of a [128, 128] PSUM tile (torch-style: ddof=1,
    eps added to std).  ``eps`` is a float or a [128, 1] AP (per-row).
    Returns a [128, 128] SBUF tile of out_dtype."""
    nc = tc.nc
    stats = scal.tile([128, 6], F32, tag="ln_stats")
    mv = scal.tile([128, 2], F32, tag="ln_mv")
    nc.vector.bn_stats(out=stats, in_=rows_ps)
    nc.vector.bn_aggr(out=mv, in_=stats)
    sd = scal.tile([128, 2], F32, tag="ln_sd")
    nc.scalar.activation(
        out=sd[:, 0:1], in_=mv[:, 1:2], func=AF.Sqrt,
        scale=float(D) / (D - 1))
    if isinstance(eps, float):
        nc.vector.tensor_scalar(
            out=sd[:, 1:2], in0=sd[:, 0:1], scalar1=eps, scalar2=None,
            op0=OP.add)
    else:
        nc.vector.tensor_scalar(
            out=sd[:, 1:2], in0=sd[:, 0:1], scalar1=eps, scalar2=None,
            op0=OP.add)
    rstd = scal.tile([128, 1], F32, tag="ln_rstd")
    nc.vector.reciprocal(out=rstd, in_=sd[:, 1:2])
    out = spool.tile([128, 128], out_dtype, tag=f"ln_out_{out_dtype}")
    nc.vector.tensor_scalar(
        out=out, in0=rows_ps, scalar1=mv[:, 0:1], scalar2=rstd,
        op0=OP.subtract, op1=OP.mult)
    return out


# ----------------------------------------------------------------------------
# host side
# ----------------------------------------------------------------------------

def _host_prep_shared(E, Dx, Dy, T):
    """Packed B32a/B32b/B16 templates (per-core slots for Vt/Vh left zero)."""
    TC = T // 128
    W16 = 2 * N + TC * 128 + TC * T + 128
    DxT = (Dx.T / X_DECAY).astype(np.float32)
    B32a = np.zeros((128, T + TC + N // 2), dtype=np.float32)
    EPSh = np.zeros((128, 2), dtype=np.float32)
    for tcn in range(TC):
        ts = tcn * 128 + np.arange(128, dtype=np.float64)
        EPSh[:, tcn] = (1e-6 * U_DECAY ** (-ts)).astype(np.float32)
    B32a[:, T + TC:] = DxT[:, 0:N // 2]
    B32b = np.ascontiguousarray(DxT[:, N // 2:])
    B16 = np.zeros((128, W16), dtype=np.float16)
    B16[:, 0:N] = Dy.reshape(C, 128, D).transpose(2, 0, 1).reshape(128, N)
    B16[:, N:2 * N] = E.reshape(D, C, 128).transpose(2, 1, 0).reshape(128, N)
    # strict-causal mask: mask[sc][i, t] = (sc*128 + i) < t
    s_all = np.arange(T)[:, None]
    t_all = np.arange(T)[None, :]
    m = (s_all < t_all).astype(np.float16)           # [s, t]
    B16[:, 2 * N + TC * 128:2 * N + TC * 128 + TC * T] = (
        m.reshape(TC, 128, T).transpose(1, 0, 2).reshape(128, TC * T))
    B16[:, 2 * N + TC * 128 + TC * T:] = np.eye(128, dtype=np.float16)
    return B32a, B32b, B16, EPSh


def _host_prep_core(B32at, B32bt, B16t, EPSht, token_emb, tokens_b, T):
    TC = T // 128
    B32a = B32at.copy()
    B16 = B16t.copy()
    V_all = token_emb[tokens_b].astype(np.float32)         # [T, 128]
    B32a[:, 0:T] = V_all.T
    decay = (U_DECAY ** (-np.arange(T, dtype=np.float64))).astype(np.float32)
    Vh_flat = (V_all * decay[:, None]).astype(np.float16)  # [T, 128]
    B16[:, 2 * N:2 * N + TC * 128] = (
        Vh_flat.reshape(TC, 128, 128).transpose(1, 0, 2).reshape(128, TC * 128))
    return dict(B32a=B32a, B32b=B32bt, B16=B16, EPS=EPSht)


_PROGRAM_CACHE = {}
RUN_KWARGS = {}      # extra kwargs forwarded to run_bass_kernel_spmd
LAST_RESULTS = None  # BassKernelResults of the most recent kernel() call


def _build(T):
    key = T
    if key in _PROGRAM_CACHE:
        return _PROGRAM_CACHE[key]
    TC = T // 128
    W32 = N + T + TC
    W16 = 2 * N + TC * 128 + TC * T + 128
    nc = bacc.Bacc("TRN2")
    ins = {
        "B32a": nc.dram_tensor(
            "B32a", [128, T + TC + N // 2], F32R, kind="ExternalInput").ap(),
        "B32b": nc.dram_tensor(
            "B32b", [128, N // 2], F32R, kind="ExternalInput").ap(),
        "EPS": nc.dram_tensor("EPS", [128, 2], F32, kind="ExternalInput").ap(),
        "B16": nc.dram_tensor("B16", [128, W16], F16, kind="ExternalInput").ap(),
    }
    outs = {
        "out": nc.dram_tensor("out", [T, D], F32, kind="ExternalOutput").ap(),
    }
    with tile.TileContext(nc) as tc:
        scan_program(tc, outs, ins, T)
    nc.compile()
    _PROGRAM_CACHE[key] = (nc, ins, outs)
    return _PROGRAM_CACHE[key]


def kernel(E, Dx, Dy, token_emb, tokens):
    from concourse.bass_utils import run_bass_kernel_spmd

    E = np.asarray(E, dtype=np.float32)
    Dx = np.asarray(Dx, dtype=np.float32)
    Dy = np.asarray(Dy, dtype=np.float32)
    token_emb = np.asarray(token_emb, dtype=np.float32)
    tokens = np.asarray(tokens)
    B, T = tokens.shape

    nc, ins, outs = _build(T)
    B32at, B32bt, B16t, EPSht = _host_prep_shared(E, Dx, Dy, T)
    in_maps = [
        _host_prep_core(B32at, B32bt, B16t, EPSht, token_emb, tokens[b], T)
        for b in range(B)
    ]

    res = run_bass_kernel_spmd(nc, in_maps, core_ids=list(range(B)), **RUN_KWARGS)
    global LAST_RESULTS
    LAST_RESULTS = res
    out = np.stack([r["out"] for r in res.results])  # [B, T, 128]
    return out.astype(np.float32)


# revision 40
# speedup vs baseline: 1.2037x; 1.0715x over previous
"""Trainium2 Bass kernel for nn_BDHGPURefStabilized.

Model (per batch element b, scan over T steps):
    v_t   = token_emb[tok_t]                         # [D]
    xt    = 0.97*x + v_t @ Dx.T                      # [N]
    xt    = xt / (sum|xt| + 1e-6)
    xt    = where(xt > 0.02*max(xt), xt, 0)
    a*    = rho @ xt                                 # fast-weight read [D]
    y     = LN(a*) @ Dy.T                            # [N]
    yt    = relu(y) * relu(xt)
    v*_t  = LN(yt @ E.T)                             # output row [D]
    rho   = 0.97*(rho + v_t (x) xt)                  # rank-1 fast-weight update

Kernel strategy (8 NeuronCores, data-parallel over batch B=8, one batch
element per core, zero collectives):

 - Split the computation into a minimal serial spine and a fully batched
   output chain.  The spine is the only true recurrence: with the
   rescaling w_t = xt_t/0.97 and host-prescaled P~ = (v @ Dx.T)/0.97,
       w_{t+1} = (w_t > 0.02 max(w_t)) * w_t / sum|w_t| + P~_{t+1}
   and the masked-normalized history is x_t = w_{t+1} - P~_{t+1}.
   Per step: two DVE free-axis reductions, two gpsimd partition_all_reduce
   ops (cross-partition max / sum, result broadcast to all partitions),
   and three DVE element-wise ops.  History (x_t and relu(x_t), fp16) is
   written by the otherwise-idle ACT engine off the critical path.
 - The output chain is a pure function of the history, so it runs once,
   batched over all T as large matmuls: G = Xh^T Xh (Gram vs history),
   strict-causal mask, A = G^T @ (0.97^{-s} v_s) with the 0.97^t factor
   folded into a per-row LayerNorm epsilon, then LN, y = LN(A) @ Dy.T,
   yt = relu(y)*relu(x), u = yt @ E.T, out = LN(u) — all LayerNorms
   batched 128 rows/op via bn_stats.  Output-path matmuls run fp16
   (PSUM accumulates fp32); the spine stays fp32.
 - This takes the Tensor-engine sequencer from ~28k instructions (the
   previous per-step formulation) to ~120, and the per-step critical
   path from ~4.5us to ~0.6us.

Output per core: [T, 128] fp32 rows; host stacks [B, T, D].
"""

from contextlib import ExitStack

import numpy as np

import concourse.bass as bass
import concourse.bacc as bacc
import concourse.tile as tile
from concourse import bass_isa, mybir

F32 = mybir.dt.float32
F32R = mybir.dt.float32r
F16 = mybir.dt.float16
AX = mybir.AxisListType
OP = mybir.AluOpType
AF = mybir.ActivationFunctionType
RED = bass_isa.ReduceOp

N, D, V = 2048, 128, 131072
C = N // 128  # 16 column-chunks of n; n = c*128 + j
U_DECAY, X_DECAY, THR = 0.97, 0.97, 0.02

# spine emission variant (selected by TimelineSim sweep)
SPINE = {
    "thr_mode": "post",     # "pre" | "post"
    "pool_order": "max_first",  # "max_first" | "sum_first"
    "red_order": "max_first",   # "max_first" | "sum_first"
    "hist": "bulk",         # "act" | "bulk"
    "keepalive": True,      # per-step PE dummy matmul (p-state warm-keeping)
    "recip_gate_ns": 1500,  # scheduling-time gate keeping recip after stt1
}


def scan_program(tc, outs, ins, T):
    nc = tc.nc
    ctx = ExitStack()
    TC = T // 128          # t-chunks (2 for T=256)
    W32 = N + T + TC       # packed f32 input width
    W16 = 2 * N + TC * 128 + TC * T + 128  # packed f16 input width

    with ctx:
        wpool = ctx.enter_context(tc.tile_pool(name="weights", bufs=1))
        spool = ctx.enter_context(tc.tile_pool(name="step", bufs=3))
        scal = ctx.enter_context(tc.tile_pool(name="scal", bufs=4))

        B32a = wpool.tile([128, T + TC + N // 2], F32R, tag="B32a")
        B32b = wpool.tile([128, N // 2], F32R, tag="B32b")
        B16 = wpool.tile([128, W16], F16, tag="B16")
        EPS = wpool.tile([128, 2], F32, tag="EPS")
        # B32 is split so the first P~ matmuls start after ~half the f32
        # input has landed
        nc.sync.dma_start(out=B32a, in_=ins["B32a"])
        nc.sync.dma_start(out=EPS, in_=ins["EPS"])
        nc.sync.dma_start(out=B32b, in_=ins["B32b"])
        nc.sync.dma_start(out=B16, in_=ins["B16"])
        Vt = B32a[:, 0:T]                          # [d, t]
        eps2 = EPS[:, 0:TC]                        # LN(A) eps per t-chunk

        def DxTc(c):
            # [d, 128] chunk c of Dx.T/0.97
            if c < C // 2:
                return B32a[:, T + TC + c * 128:T + TC + (c + 1) * 128]
            return B32b[:, (c - C // 2) * 128:(c - C // 2 + 1) * 128]
        DyTr = B16[:, 0:N].rearrange("p (c j) -> p c j", c=C)      # [d,(c,j)]
        ETr = B16[:, N:2 * N].rearrange("p (c j) -> p c j", c=C)   # [j,(c,d)]
        Vh = B16[:, 2 * N:2 * N + TC * 128].rearrange(
            "p (s j) -> p s j", s=TC)                              # [s,(sc,d)]
        mask16 = B16[:, 2 * N + TC * 128:2 * N + TC * 128 + TC * T].rearrange(
            "p (s j) -> p s j", s=TC)                              # [s,(sc,t)]
        idn16 = B16[:, 2 * N + TC * 128 + TC * T:]                 # [128,128] I

        # persistent SBUF state.  Spine state is laid out with one column per
        # step (zero buffer reuse): every spine instruction then carries at
        # most ONE semaphore wait, so bacc's multi-wait splitting never
        # inserts SEQ-blocking EventSemaphore instructions into the loop.
        P_sb = wpool.tile([128, T, C], F32, tag="P_sb")    # P~ = v@Dx.T/0.97
        Xh = wpool.tile([128, C, T], F16, tag="Xh")        # x_t history
        W16h = wpool.tile([128, C, T], F16, tag="W16h")    # relu(x_t) history
        Wsp = wpool.tile([128, T, C], F32, tag="Wsp")      # spine state u_t
        Ysp = wpool.tile([128, T, C], F32, tag="Ysp")      # masked u_t
        red2T = wpool.tile([128, 2, T], F32, tag="red2T")  # partial reduces
        ar2T = wpool.tile([128, 2, T], F32, tag="ar2T")    # all-reduced S, m
        thrT = wpool.tile([128, 1, T], F32, tag="thrT")
        invT = wpool.tile([128, 1, T], F32, tag="invT")
        rall = wpool.tile([128, T], F32, tag="rall")       # 1/S_t (bulk)

        # ---- PE warm-up: ~3us of junk matmuls overlapping the input DMA
        # ramps the tensor engine to full clock before the fp32 P~ matmuls
        # (cold fp32 matmuls run at 2-4x the cycle time).
        warm = wpool.tile([128, 256], F16, tag="warm")
        nc.vector.memset(warm, 0.0)
        # dummy Sqrt+Relu so the ACT function-table loads happen here, in
        # DMA dead time, instead of mid-way through the output chain
        aw = wpool.tile([1, 2], F32, tag="actwarm")
        nc.vector.memset(aw, 1.0)
        nc.scalar.activation(out=aw[:, 1:2], in_=aw[:, 1:2], func=AF.Relu)
        nc.scalar.activation(out=aw[:, 0:1], in_=aw[:, 0:1], func=AF.Sqrt)
        with tc.tile_pool(name="pwarm", bufs=2, space="PSUM") as pwarm:
            for i in range(14):
                w_ps = pwarm.tile([128, 256], F32, tag="w")
                nc.tensor.matmul(
                    w_ps, warm[:, 0:128], warm, start=True, stop=True)

        # ---- P~ = DxT~ @ V (device-side, fp32) ----
        # PSUM evacuation on DVE (not ACT) so the spine's P_sb readers wait
        # on the DVE semaphore only (coalesces with their other DVE waits).
        with tc.tile_pool(name="psetup", bufs=2, space="PSUM") as psetup:
            for c in range(C):
                p_ps = psetup.tile([128, T], F32, tag="pp")
                nc.tensor.matmul(
                    p_ps, DxTc(c), Vt,
                    start=True, stop=True,
                )
                nc.vector.tensor_copy(P_sb[:, :, c], p_ps)

        # ---- serial spine ----
        # The 0.02 threshold scale is applied to the PER-PARTITION partial
        # maxima before the cross-partition allreduce (exact: fl(0.02*x) is
        # monotone, so max_p fl(0.02*m_p) == fl(0.02*max_p m_p)), so the
        # Pool allreduce returns thr directly and the mask STT carries a
        # single Pool-sem wait.  Pool stays PartitionAllReduce-only (any
        # standard gpsimd op in the loop would force a Q7 library reload
        # per step).  Histories are normalized in bulk after the loop.
        pkeep = ctx.enter_context(
            tc.tile_pool(name="pkeep", bufs=2, space="PSUM"))
        for t in range(T):
            u = Wsp[:, t, :] if t > 0 else P_sb[:, 0, :]
            pmax = red2T[:, 1, t:t + 1]
            psum = red2T[:, 0, t:t + 1]
            armax = ar2T[:, 1, t:t + 1]
            arsum = ar2T[:, 0, t:t + 1]
            thrp = thrT[:, 0, t:t + 1]
            invs = invT[:, 0, t:t + 1]
            y = Ysp[:, t, :]

            def emit_redmax():
                nc.vector.tensor_reduce(
                    out=pmax, in_=u, axis=AX.X, op=OP.max)
                if SPINE["thr_mode"] == "pre":
                    # pre-scale by 0.02 (exact: fl(0.02*x) is monotone) so
                    # the Pool allreduce returns thr directly
                    nc.vector.tensor_scalar(
                        out=thrp, in0=pmax, scalar1=float(THR),
                        scalar2=None, op0=OP.mult)

            def emit_redsum():
                nc.vector.tensor_reduce(
                    out=psum, in_=u, axis=AX.X, op=OP.add,
                    apply_absolute_value=True)

            def emit_par_max():
                nc.gpsimd.partition_all_reduce(
                    armax, thrp if SPINE["thr_mode"] == "pre" else pmax,
                    128, RED.max)

            def emit_par_sum():
                nc.gpsimd.partition_all_reduce(arsum, psum, 128, RED.add)

            if SPINE["red_order"] == "max_first":
                emit_redmax()
                emit_redsum()
            else:
                emit_redsum()
                emit_redmax()
            if SPINE["pool_order"] == "max_first":
                emit_par_max()
                emit_par_sum()
            else:
                emit_par_sum()
                emit_par_max()
            if SPINE["thr_mode"] == "post":
                nc.vector.tensor_scalar(
                    out=thrp, in0=armax, scalar1=float(THR),
                    scalar2=None, op0=OP.mult)
                thr_ap = thrp
            else:
                thr_ap = armax
            nc.vector.scalar_tensor_tensor(
                out=y, in0=u, scalar=thr_ap, in1=u,
                op0=OP.is_gt, op1=OP.mult)
            # gate the reciprocal's *scheduling* time past stt1's so the
            # Tile scheduler doesn't hoist it ahead (its PAR-sum wait would
            # then stall stt1 in the in-order DVE stream)
            gate = SPINE.get("recip_gate_ns")
            if gate:
                with tc.tile_wait_until((10000 + t * gate) / 1e6):
                    nc.vector.reciprocal(out=invs, in_=arsum)
            else:
                nc.vector.reciprocal(out=invs, in_=arsum)
            if t + 1 < T:
                nc.vector.scalar_tensor_tensor(
                    out=Wsp[:, t + 1, :], in0=y, scalar=invs,
                    in1=P_sb[:, t + 1, :], op0=OP.mult, op1=OP.add)
            if SPINE.get("keepalive"):
                # tiny per-step matmul keeps the PE p-state ramp alive so
                # the post-spine batched matmuls start at full clock
                k_ps = pkeep.tile([1, 2], F32, tag="k")
                nc.tensor.matmul(
                    k_ps, red2T[:, 0, t:t + 1], red2T[:, :, t],
                    start=True, stop=True)
            if SPINE["hist"] == "act":
                nc.scalar.activation(
                    out=Xh[:, :, t], in_=y, func=AF.Copy, scale=invs)
                nc.scalar.activation(
                    out=W16h[:, :, t], in_=y, func=AF.Relu, scale=invs)
        if SPINE["hist"] == "bulk":
            nc.vector.reciprocal(out=rall, in_=ar2T[:, 0, :])
            for c in range(C):
                # split the 16 normalization multiplies between DVE and the
                # otherwise-idle Pool engine (standard-lib ops are fine here,
                # the spine's allreduce stream is over)
                eng = nc.gpsimd if c % 3 == 2 else nc.vector
                eng.tensor_tensor(
                    out=Xh[:, c, :], in0=Ysp[:, :, c], in1=rall, op=OP.mult)
                nc.scalar.activation(
                    out=W16h[:, c, :], in_=Xh[:, c, :], func=AF.Relu)

        # ---- batched output chain ----
        # G[s, t] = x_s . x_t   (strict-causal masked, fp16)
        G16 = wpool.tile([128, TC, T], F16, tag="G16")
        with tc.tile_pool(name="pG", bufs=2, space="PSUM") as pG:
            for sc in range(TC):
                g_ps = pG.tile([128, T], F32, tag="g")
                for c in range(C):
                    nc.tensor.matmul(
                        g_ps, Xh[:, c, sc * 128:(sc + 1) * 128], Xh[:, c, :],
                        start=(c == 0), stop=(c == C - 1),
                    )
                nc.vector.tensor_tensor(
                    out=G16[:, sc, :], in0=g_ps, in1=mask16[:, sc, :],
                    op=OP.mult)

        # A[t, d] = sum_s G[s, t] Vh[s, d];  LnA = rowwise-LN(A, eps_t)
        LnAT = wpool.tile([128, T], F16, tag="LnAT")     # [d, t]
        with tc.tile_pool(name="pA", bufs=2, space="PSUM") as pA, \
                tc.tile_pool(name="pT", bufs=2, space="PSUM") as pT:
            for tcn in range(TC):
                a_ps = pA.tile([128, 128], F32, tag="a")
                for sc in range(TC):
                    nc.tensor.matmul(
                        a_ps, G16[:, sc, tcn * 128:(tcn + 1) * 128],
                        Vh[:, sc, :], start=(sc == 0), stop=(sc == TC - 1),
                    )
                lnA = _layernorm_rows(
                    tc, spool, scal, a_ps, eps2[:, tcn:tcn + 1], F16)
                t_ps = pT.tile([128, 128], F16, tag="t")
                nc.tensor.transpose(t_ps, lnA, idn16)
                nc.scalar.copy(LnAT[:, tcn * 128:(tcn + 1) * 128], t_ps)

        # yT[n, t] = Dy @ LnA^T;  yt = relu(yT) * relu(x)  (fp16)
        yt_sb = wpool.tile([128, C, T], F16, tag="yt")
        with tc.tile_pool(name="pY", bufs=3, space="PSUM") as pY:
            for c in range(C):
                y_ps = pY.tile([128, T], F32, tag="y")
                nc.tensor.matmul(y_ps, DyTr[:, c, :], LnAT,
                                 start=True, stop=True)
                if c % 3 == 0:
                    # DVE: fused relu+mask-multiply straight from PSUM
                    nc.vector.scalar_tensor_tensor(
                        out=yt_sb[:, c, :], in0=y_ps, scalar=0.0,
                        in1=W16h[:, c, :], op0=OP.max, op1=OP.mult)
                else:
                    # ACT evacuates with relu; DVE/Pool multiply fp16
                    ry = spool.tile([128, T], F16, tag="ry")
                    nc.scalar.activation(out=ry, in_=y_ps, func=AF.Relu)
                    eng = nc.vector if c % 3 == 1 else nc.gpsimd
                    eng.tensor_tensor(
                        out=yt_sb[:, c, :], in0=ry, in1=W16h[:, c, :],
                        op=OP.mult)

        # u[t, d] = sum_n yt[n, t] E[d, n];  out = rowwise-LN(u)
        with tc.tile_pool(name="pU", bufs=2, space="PSUM") as pU:
            for tcn in range(TC):
                u_ps = pU.tile([128, 128], F32, tag="u")
                for c in range(C):
                    nc.tensor.matmul(
                        u_ps, yt_sb[:, c, tcn * 128:(tcn + 1) * 128],
                        ETr[:, c, :], start=(c == 0), stop=(c == C - 1),
                    )
                o_sb = _layernorm_rows(tc, spool, scal, u_ps, 1e-6, F32)
                nc.sync.dma_start(
                    out=outs["out"][tcn * 128:(tcn + 1) * 128, :], in_=o_sb)


def _layernorm_rows(tc, spool, scal, rows_ps, eps, out_dtype):
    """Row-wise LayerNorm of a [128, 128] PSUM tile (torch-style: ddof=1,
    eps added to std).  ``eps`` is a float or a [128, 1] AP (per-row).
    Returns a [128, 128] SBUF tile of out_dtype."""
    nc = tc.nc
    stats = scal.tile([128, 6], F32, tag="ln_stats")
    mv = scal.tile([128, 2], F32, tag="ln_mv")
    nc.vector.bn_stats(out=stats, in_=rows_ps)
    nc.vector.bn_aggr(out=mv, in_=stats)
    sd = scal.tile([128, 2], F32, tag="ln_sd")
    nc.scalar.activation(
        out=sd[:, 0:1], in_=mv[:, 1:2], func=AF.Sqrt,
        scale=float(D) / (D - 1))
    if isinstance(eps, float):
        nc.vector.tensor_scalar(
            out=sd[:, 1:2], in0=sd[:, 0:1], scalar1=eps, scalar2=None,
            op0=OP.add)
    else:
        nc.vector.tensor_scalar(
            out=sd[:, 1:2], in0=sd[:, 0:1], scalar1=eps, scalar2=None,
            op0=OP.add)
    rstd = scal.tile([128, 1], F32, tag="ln_rstd")
    nc.vector.reciprocal(out=rstd, in_=sd[:, 1:2])
    out = spool.tile([128, 128], out_dtype, tag=f"ln_out_{out_dtype}")
    nc.vector.tensor_scalar(
        out=out, in0=rows_ps, scalar1=mv[:, 0:1], scalar2=rstd,
        op0=OP.subtract, op1=OP.mult)
    return out


# ----------------------------------------------------------------------------
# host side
# ----------------------------------------------------------------------------

def _host_prep_shared(E, Dx, Dy, T):
    """Packed B32a/B32b/B16 templates (per-core slots for Vt/Vh left zero)."""
    TC = T // 128
    W16 = 2 * N + TC * 128 + TC * T + 128
    DxT = (Dx.T / X_DECAY).astype(np.float32)
    B32a = np.zeros((128, T + TC + N // 2), dtype=np.float32)
    EPSh = np.zeros((128, 2), dtype=np.float32)
    for tcn in range(TC):
        ts = tcn * 128 + np.arange(128, dtype=np.float64)
        EPSh[:, tcn] = (1e-6 * U_DECAY ** (-ts)).astype(np.float32)
    B32a[:, T + TC:] = DxT[:, 0:N // 2]
    B32b = np.ascontiguousarray(DxT[:, N // 2:])
    B16 = np.zeros((128, W16), dtype=np.float16)
    B16[:, 0:N] = Dy.reshape(C, 128, D).transpose(2, 0, 1).reshape(128, N)
    B16[:, N:2 * N] = E.reshape(D, C, 128).transpose(2, 1, 0).reshape(128, N)
    # strict-causal mask: mask[sc][i, t] = (sc*128 + i) < t
    s_all = np.arange(T)[:, None]
    t_all = np.arange(T)[None, :]
    m = (s_all < t_all).astype(np.float16)           # [s, t]
    B16[:, 2 * N + TC * 128:2 * N + TC * 128 + TC * T] = (
        m.reshape(TC, 128, T).transpose(1, 0, 2).reshape(128, TC * T))
    B16[:, 2 * N + TC * 128 + TC * T:] = np.eye(128, dtype=np.float16)
    return B32a, B32b, B16, EPSh


def _host_prep_core(B32at, B32bt, B16t, EPSht, token_emb, tokens_b, T):
    TC = T // 128
    B32a = B32at.copy()
    B16 = B16t.copy()
    V_all = token_emb[tokens_b].astype(np.float32)         # [T, 128]
    B32a[:, 0:T] = V_all.T
    decay = (U_DECAY ** (-np.arange(T, dtype=np.float64))).astype(np.float32)
    Vh_flat = (V_all * decay[:, None]).astype(np.float16)  # [T, 128]
    B16[:, 2 * N:2 * N + TC * 128] = (
        Vh_flat.reshape(TC, 128, 128).transpose(1, 0, 2).reshape(128, TC * 128))
    return dict(B32a=B32a, B32b=B32bt, B16=B16, EPS=EPSht)


_PROGRAM_CACHE = {}
RUN_KWARGS = {}      # extra kwargs forwarded to run_bass_kernel_spmd
LAST_RESULTS = None  # BassKernelResults of the most recent kernel() call


def _build(T):
    key = T
    if key in _PROGRAM_CACHE:
        return _PROGRAM_CACHE[key]
    TC = T // 128
    W32 = N + T + TC
    W16 = 2 * N + TC * 128 + TC * T + 128
    nc = bacc.Bacc("TRN2")
    ins = {
        "B32a": nc.dram_tensor(
            "B32a", [128, T + TC + N // 2], F32R, kind="ExternalInput").ap(),
        "B32b": nc.dram_tensor(
            "B32b", [128, N // 2], F32R, kind="ExternalInput").ap(),
        "EPS": nc.dram_tensor("EPS", [128, 2], F32, kind="ExternalInput").ap(),
        "B16": nc.dram_tensor("B16", [128, W16], F16, kind="ExternalInput").ap(),
    }
    outs = {
        "out": nc.dram_tensor("out", [T, D], F32, kind="ExternalOutput").ap(),
    }
    with tile.TileContext(nc) as tc:
        scan_program(tc, outs, ins, T)
    nc.compile()
    _PROGRAM_CACHE[key] = (nc, ins, outs)
    return _PROGRAM_CACHE[key]


def kernel(E, Dx, Dy, token_emb, tokens):
    from concourse.bass_utils import run_bass_kernel_spmd

    E = np.asarray(E, dtype=np.float32)
    Dx = np.asarray(Dx, dtype=np.float32)
    Dy = np.asarray(Dy, dtype=np.float32)
    token_emb = np.asarray(token_emb, dtype=np.float32)
    tokens = np.asarray(tokens)
    B, T = tokens.shape

    nc, ins, outs = _build(T)
    B32at, B32bt, B16t, EPSht = _host_prep_shared(E, Dx, Dy, T)
    in_maps = [
        _host_prep_core(B32at, B32bt, B16t, EPSht, token_emb, tokens[b], T)
        for b in range(B)
    ]

    res = run_bass_kernel_spmd(nc, in_maps, core_ids=list(range(B)), **RUN_KWARGS)
    global LAST_RESULTS
    LAST_RESULTS = res
    out = np.stack([r["out"] for r in res.results])  # [B, T, 128]
    return out.astype(np.float32)
